# revision 11
# baseline (speedup 1.0000x reference)
"""Tensor-parallel MultiHeadAttention (QKV + RoPE + GQA causal SDPA + dense)
for 8 Trainium2 NeuronCores — bf16, kv-projection-deduplicated edition (v5).

Sharding (TP as in TPMultiHeadAttention): core d owns query heads {2d, 2d+1}
and kv head d//2. The kv projection is deduplicated across each core pair:
core 2g projects only k_g, core 2g+1 only v_g (128 rows instead of 256);
the pair exchanges raw kv chunks through DRAM bounce buffers + a pairwise
AllGather (replica_groups=[[0,1],[2,3],[4,5],[6,7]]), removing 1/4 of the
QKV matmul columns (~13.7us of PE streaming per core). AllGather output is
flat-concat ordered [k_g, v_g]; both cores read both halves back uniformly
(SPMD-safe). RoPE for k and the XBAR v transposes run after the exchange.

Collective latency management (measured: ~25us first call incl. NRT init +
core launch skew, ~8-12us steady-state doorbell->done):
  - a dummy warmup AllGather on uninitialized DRAM scratch is issued as the
    first gpsimd instruction (~7us), absorbing init cost + skew while the
    qkv stream runs;
  - all 4 collectives stay on the gpsimd queue (NRT ordering requirement)
    but the DRAM->SBUF readbacks are emitted AFTER the next chunk's
    doorbell, so the in-order queue pipelines the collectives instead of
    serializing doorbell(c+1) behind readback(c);
  - cin DMAs ride the scalar HWDGE ring (trigger directly after the kvsb
    psum->SBUF copy on the same engine); the 1MB rope tables moved off the
    gpsimd SWDGE ring (they delayed the first cin by ~8us) to the scalar
    ring after chunk 0's weights; wd follows after chunk 1; v transposes
    ride the scalar ring too, keeping the sync ring pure x-stream + output.
  - attention is scheduled one phase later than data-flow allows at zero
    latency: attn(0) in chunks 2-3, attn(1) in chunk 3 + B0, attn(2) in
    B1, attn(3) in B2/B3, each against the dense filler of an earlier
    chunk.

Phase B: exp on ScalarE (1.12us/step) is the serializer; dense psum copies
split vector/scalar per eo; the softmax finalization of the LAST chunk is
per-128-column st-slice (only needs ctx/acc columns final after ctx matmul
j=12+st), so dense(3, st) starts while the attention drain still runs.
All matmul operands bf16; PSUM fp32. Host sums the 8 bf16 partials.
"""

import numpy as np
import ml_dtypes

B, S, E = 1, 2048, 2048
H, KVH, D = 16, 4, 128
NCORES = 8
P = 128
FD = 512            # matmul moving free dim == one fp32 PSUM bank
NE = E // P         # 16 contraction tiles over the embedding dim
NG = 4              # eo-groups of 4
NSC = S // FD       # 4 sequence chunks
NST = S // P        # 16 sequence tiles
FLOC = 3 * P        # local fused qkv rows per core (2 q heads + k XOR v)
ROPE_BASE = 10000.0
DIAG_START = (0, 128, 256, 384)
BF = ml_dtypes.bfloat16
GROUPS = [[0, 1], [2, 3], [4, 5], [6, 7]]

LAST_RESULT = None
_BASS_CACHE = None


def _rope_tables():
    inv = 1.0 / (ROPE_BASE ** (np.arange(0, D, 2, dtype=np.float64) / D))
    t = np.arange(S, dtype=np.float64)
    freqs = np.outer(t, inv)
    emb = np.concatenate([freqs, freqs], axis=-1)  # [S, D]
    return np.cos(emb), np.sin(emb)


def _host_constants():
    cos, sin = _rope_tables()
    cos_ds = np.ascontiguousarray(cos.T)  # [D, S]
    sin_ds = np.ascontiguousarray(sin.T)
    # sign-folded sin for the partition-shifted rotate-half:
    # tt[d] = qt[(d+64)%128] * sg[d],  sg = [-sin[:64]; +sin[64:]]
    sg = np.concatenate([-sin_ds[:64], sin_ds[64:]], axis=0)
    r_idx = np.arange(P)[:, None]
    c_idx = np.arange(P)[None, :]
    tri = (r_idx <= c_idx).astype(np.float64)
    return {
        "cosr": cos_ds.astype(BF),
        "sgsin": sg.astype(BF),
        "trim": tri.astype(BF),
        "ones": np.ones((P, 1), np.float64).astype(BF),
    }


def _build_bass():
    import concourse.mybir as mybir
    import concourse.tile as tile
    from concourse import bacc

    f32 = mybir.dt.float32
    bf16 = mybir.dt.bfloat16
    Exp = mybir.ActivationFunctionType.Exp

    nc = bacc.Bacc(None, target_bir_lowering=False, name="mha_tp8_v5")
    xG = nc.dram_tensor("xG", [NSC, NG, P, 4, FD], bf16, kind="ExternalInput")
    wG = nc.dram_tensor("wG", [NG, P, 4, FLOC], bf16, kind="ExternalInput")
    wdG = nc.dram_tensor("wdG", [P, 2, S], bf16, kind="ExternalInput")
    cosr = nc.dram_tensor("cosr", [P, S], bf16, kind="ExternalInput")
    sgsin = nc.dram_tensor("sgsin", [P, S], bf16, kind="ExternalInput")
    trim = nc.dram_tensor("trim", [P, P], bf16, kind="ExternalInput")
    ones = nc.dram_tensor("ones", [P, 1], bf16, kind="ExternalInput")
    out = nc.dram_tensor("out", [NSC, 4, P, 4, FD], bf16, kind="ExternalOutput")

    with tile.TileContext(nc) as tc:
        with tc.tile_pool(name="const", bufs=1) as const, \
             tc.tile_pool(name="ps_ctx", bufs=2, space="PSUM") as ps_ctx, \
             tc.tile_pool(name="xs_p", bufs=5) as xpool, \
             tc.tile_pool(name="rtmp", bufs=3) as rtmp, \
             tc.tile_pool(name="kv_p", bufs=2) as kvp, \
             tc.tile_pool(name="pt_p", bufs=8) as ptp, \
             tc.tile_pool(name="acc_p", bufs=2) as accp, \
             tc.tile_pool(name="dn_p", bufs=2) as dnp, \
             tc.tile_pool(name="ctx_p", bufs=3) as ctxp, \
             tc.tile_pool(name="out_p", bufs=3) as outp, \
             tc.tile_pool(name="dram", bufs=2, space="DRAM") as dram:
            w_sb = const.tile([P, NE, FLOC], bf16, name="w_sb")
            cq = const.tile([P, S], bf16, name="cq")
            sg = const.tile([P, S], bf16, name="sg")
            mk = const.tile([P, P], bf16, name="mk")
            wd_sb = const.tile([P, 2, S], bf16, name="wd_sb")
            qr = const.tile([P, 2, S], bf16, name="qr")
            kr = const.tile([P, S], bf16, name="kr")
            vT = const.tile([P, S], bf16, name="vT")
            vn = const.tile([P, NST, P], bf16, name="vn")
            on = const.tile([P, 1], bf16, name="on")
            warm = const.tile([P, 8], bf16, name="warm")

            # ---- shared attention machinery (paired heads per j-tile) ----
            st_ = {}          # per-chunk attention state
            all_csb = {}      # (c, h) -> [P, FD] tile, or (c, h, st) -> [P, P]

            def attn_begin(c):
                two = c >= 1
                ctxps = [
                    ps_ctx.tile([P, FD], f32, tag="ctx", name=f"ctx_{c}_{h}")
                    for h in range(2)
                ]
                accs = [
                    accp.tile([P, 2, FD], bf16, tag=f"acc{ch}", name=f"acc_{c}_{ch}")
                    for ch in range(2 if two else 1)
                ]
                st_[c] = (ctxps, accs, two)

            def attn_step(c, j, sdpool, split):
                ctxps, accs, two = st_[c]
                o = j - 4 * c
                so = DIAG_START[o] if o >= 0 else 0
                n = FD - so
                pt = ptp.tile([P, 2, FD], bf16, tag="pt", name=f"pt_{c}_{j}")
                if split:
                    # phase A: one psum bank per head so sd bufs=3 pipelines
                    for h in range(2):
                        sd = sdpool.tile([P, FD], f32, tag="sd",
                                         name=f"sd_{c}_{j}_{h}")
                        nc.tensor.matmul(
                            sd[:, :n],
                            lhsT=kr[:, j * P:(j + 1) * P],
                            rhs=qr[:, h, c * FD + so:(c + 1) * FD],
                            start=True, stop=True,
                        )
                        nc.scalar.activation(pt[:, h, :n], sd[:, :n], Exp)
                else:
                    sd = sdpool.tile([P, 2, FD], f32, tag="sd", name=f"sd_{c}_{j}")
                    for h in range(2):
                        nc.tensor.matmul(
                            sd[:, h, :n],
                            lhsT=kr[:, j * P:(j + 1) * P],
                            rhs=qr[:, h, c * FD + so:(c + 1) * FD],
                            start=True, stop=True,
                        )
                    nc.scalar.activation(pt[:, :, :n], sd[:, :, :n], Exp)
                if o >= 0:
                    for h in range(2):
                        nc.vector.tensor_mul(pt[:, h, :P], pt[:, h, :P], mk)
                acc = accs[j % 2 if two else 0]
                if j < (2 if two else 1):
                    nc.vector.tensor_copy(acc, pt)
                else:
                    nc.vector.tensor_add(acc[:, :, so:], acc[:, :, so:], pt[:, :, :n])
                return (j, pt, so, n)

            def attn_ctx(c, ent):
                ctxps, _, _ = st_[c]
                nj = 4 * c + 4
                j, pt, so, n = ent
                for h in range(2):
                    nc.tensor.matmul(
                        ctxps[h][:, so:],
                        lhsT=vn[:, j, :],
                        rhs=pt[:, h, :n],
                        start=(j == 0), stop=(j == nj - 1),
                    )

            def attn_tail(c, sdpool, ptag="sd"):
                # whole-chunk softmax finalize (chunks 0..2)
                ctxps, accs, two = st_[c]
                crs = []
                for h in range(2):
                    cr = ctxp.tile([P, FD], bf16, tag=f"cr{h}", name=f"cr_{c}_{h}")
                    nc.scalar.copy(cr, ctxps[h])
                    crs.append(cr)
                for h in range(2):
                    rp = sdpool.tile([1, FD], f32, tag=ptag, name=f"rp_{c}_{h}")
                    nc.tensor.matmul(rp, lhsT=on, rhs=accs[0][:, h, :],
                                     start=True, stop=not two)
                    if two:
                        nc.tensor.matmul(rp, lhsT=on, rhs=accs[1][:, h, :],
                                         start=False, stop=True)
                    rec = dnp.tile([1, FD], f32, tag=f"rec{h}", name=f"rec_{c}_{h}")
                    nc.vector.reciprocal_approx_fast(rec, rp)
                    rb = dnp.tile([P, FD], f32, tag=f"rb{h}", name=f"rb_{c}_{h}")
                    nc.gpsimd.partition_broadcast(rb, rec)
                    ct = ctxp.tile([P, FD], bf16, tag=f"ct{h}", name=f"csb_{c}_{h}")
                    nc.vector.tensor_mul(ct, crs[h], rb)
                    all_csb[(c, h)] = ct

            def fin_st(c, stt, rppool, ptag):
                # per-st-slice finalize for the last chunk: only needs
                # ctx/acc columns that are final after ctx matmul j=4c+stt
                ctxps, accs, two = st_[c]
                r = slice(stt * P, (stt + 1) * P)
                for h in range(2):
                    cr = ctxp.tile([P, P], bf16, tag=f"crs{h}",
                                   name=f"crs_{c}_{stt}_{h}")
                    nc.scalar.copy(cr, ctxps[h][:, r])
                    rp = rppool.tile([1, P], f32, tag=ptag,
                                     name=f"rps_{c}_{stt}_{h}")
                    nc.tensor.matmul(rp, lhsT=on, rhs=accs[0][:, h, r],
                                     start=True, stop=not two)
                    if two:
                        nc.tensor.matmul(rp, lhsT=on, rhs=accs[1][:, h, r],
                                         start=False, stop=True)
                    rec = dnp.tile([1, P], f32, tag=f"recs{h}",
                                   name=f"recs_{c}_{stt}_{h}")
                    nc.vector.reciprocal_approx_fast(rec, rp)
                    rb = dnp.tile([P, P], f32, tag=f"rbs{h}",
                                  name=f"rbs_{c}_{stt}_{h}")
                    nc.gpsimd.partition_broadcast(rb, rec)
                    ct = ctxp.tile([P, P], bf16, tag=f"cts{h}",
                                   name=f"cts_{c}_{stt}_{h}")
                    nc.vector.tensor_mul(ct, cr, rb)
                    all_csb[(c, h, stt)] = ct

            # ---- kv exchange: psum -> SBUF -> DRAM -> AllGather -> SBUF ----
            kv_parts = {}
            cc_bufs = {}

            def kv_begin(sc, kv_ps):
                # kvsb copy + cin DMA on the scalar ring (HWDGE, trigger
                # directly after the copy on the same engine); doorbell on
                # the gpsimd queue
                kvsb = kvp.tile([P, FD], bf16, tag="kvsb", name=f"kvsb_{sc}")
                nc.scalar.copy(kvsb, kv_ps)
                cin = dram.tile([P, FD], bf16, tag="cin", name=f"cin_{sc}")
                cout = dram.tile([2, P, FD], bf16, tag="cout", name=f"cout_{sc}")
                nc.scalar.dma_start(cin, kvsb)
                nc.gpsimd.collective_compute(
                    "AllGather",
                    mybir.AluOpType.bypass,
                    replica_groups=GROUPS,
                    ins=[cin.opt()],
                    outs=[cout.opt()],
                )
                cc_bufs[sc] = cout

            def kv_finish(sc):
                # readbacks on gpsimd AFTER the next doorbell so the
                # in-order queue pipelines the collectives
                ssl = slice(sc * FD, (sc + 1) * FD)
                cout = cc_bufs[sc]
                kraw = kvp.tile([P, FD], bf16, tag="kraw", name=f"kraw_{sc}")
                nc.gpsimd.dma_start(kraw, cout[0])
                nc.gpsimd.dma_start(vT[:, ssl], cout[1])
                kv_parts[sc] = kraw

            def rope_arith(dst, src, ssl):
                ts = rtmp.tile([P, FD], bf16, tag="ts", name="ts")
                nc.vector.tensor_copy(ts[0:64, :], src[64:128, :])
                nc.vector.tensor_copy(ts[64:128, :], src[0:64, :])
                tt = rtmp.tile([P, FD], bf16, tag="tt", name="tt")
                nc.vector.tensor_mul(tt, ts, sg[:, ssl])
                nc.vector.tensor_mul(dst, src, cq[:, ssl])
                nc.vector.tensor_add(dst, dst, tt)

            qts = {}

            def rope_q_copy(sc, psums):
                qt0 = rtmp.tile([P, FD], bf16, tag="qt", name=f"qt_{sc}_0")
                nc.vector.tensor_copy(qt0, psums[0])
                qt1 = rtmp.tile([P, FD], bf16, tag="qt", name=f"qt_{sc}_1")
                nc.scalar.copy(qt1, psums[1])
                qts[sc] = (qt0, qt1)

            def rope_q_arith(sc, f):
                ssl = slice(sc * FD, (sc + 1) * FD)
                rope_arith(qr[:, f, ssl], qts[sc][f], ssl)

            def rope_k(sc):
                ssl = slice(sc * FD, (sc + 1) * FD)
                rope_arith(kr[:, ssl], kv_parts[sc], ssl)

            def vn_t(jt):
                # v transposes ride the scalar ring (sync stays pure x/out)
                nc.scalar.dma_start_transpose(vn[:, jt, :],
                                              vT[:, jt * P:(jt + 1) * P])

            pend = {}
            pendB = []

            # ---- Phase A: QKV (dedup) + exchanges, attn(0/1) trickled ----
            with tc.tile_pool(name="ps_qkv", bufs=1, space="PSUM") as pqkv, \
                 tc.tile_pool(name="ps_sA", bufs=3, space="PSUM") as psA:
                # dummy warmup collective: absorbs NRT first-call init +
                # inter-core launch skew under the qkv stream. Contents of
                # the uninitialized DRAM scratch are irrelevant (bypass).
                wu_in = dram.tile([P, 2], bf16, tag="wu_i", name="wu_in")
                wu_out = dram.tile([2, P, 2], bf16, tag="wu_o", name="wu_out")
                nc.gpsimd.collective_compute(
                    "AllGather",
                    mybir.AluOpType.bypass,
                    replica_groups=GROUPS,
                    ins=[wu_in.opt()],
                    outs=[wu_out.opt()],
                )
                # small tables + ucode warm on gpsimd; big tables (cq/sg)
                # load later on the scalar ring
                nc.gpsimd.dma_start(mk, trim[:, :])
                nc.gpsimd.partition_broadcast(warm, mk[0:1, 0:8])
                nc.gpsimd.dma_start(on, ones[:, :])

                qkv_ps = {}

                def qkv_units(sc):
                    """16 units; yields after each unit's 3 matmuls (~650ns)."""
                    psums = [
                        pqkv.tile([P, FD], f32, tag=f"qkv{f}", name=f"ps_qkv{f}_{sc}")
                        for f in range(3)
                    ]
                    qkv_ps[sc] = psums
                    for g in range(NG):
                        fine = sc == 0 and g == 0
                        if sc == 0 and not fine:
                            nc.scalar.dma_start(w_sb[:, 4 * g:4 * g + 4, :], wG[g])
                        xs = xpool.tile([P, 4, FD], bf16, tag="xs", name=f"xs_{sc}_{g}")
                        if not fine:
                            nc.sync.dma_start(xs, xG[sc, g])
                        for j in range(4):
                            if fine:
                                # 96KB w / 128KB x pieces so the first
                                # matmuls aren't starved
                                nc.scalar.dma_start(w_sb[:, j, :], wG[0, :, j, :])
                                nc.sync.dma_start(xs[:, j, :], xG[0, 0, :, j, :])
                            eo = 4 * g + j
                            # f-order (kv, q0, q1): kv psum completes first
                            for f in (2, 0, 1):
                                nc.tensor.matmul(
                                    psums[f],
                                    lhsT=w_sb[:, eo, f * P:(f + 1) * P],
                                    rhs=xs[:, j, :],
                                    start=(eo == 0),
                                    stop=(eo == NE - 1),
                                )
                            yield

                def SA(c, j):
                    pend[(c, j)] = attn_step(c, j, psA, split=True)

                def XA(c, j):
                    attn_ctx(c, pend.pop((c, j)))

                # chunk 0
                for _ in qkv_units(0):
                    pass
                kv_begin(0, qkv_ps[0][2])
                # rope tables on the scalar ring once chunk 0's w is done
                nc.scalar.dma_start(cq, cosr[:, :])
                nc.scalar.dma_start(sg, sgsin[:, :])
                rope_q_copy(0, qkv_ps[0])
                rope_q_arith(0, 0)
                rope_q_arith(0, 1)
                # chunk 1
                for _ in qkv_units(1):
                    pass
                kv_begin(1, qkv_ps[1][2])
                kv_finish(0)
                nc.scalar.dma_start(wd_sb, wdG[:, :, :])
                rope_q_copy(1, qkv_ps[1])
                # chunk 2: rope(1) arith, rope_k(0), attn(0) j0-1 late
                attn_begin(0)
                sched2 = {
                    2: [lambda: rope_q_arith(1, 0)],
                    8: [lambda: rope_q_arith(1, 1)],
                    10: [lambda: rope_k(0)],
                    11: [lambda: vn_t(0)],
                    12: [lambda: SA(0, 0)],
                    13: [lambda: vn_t(1)],
                    14: [lambda: SA(0, 1)],
                    15: [lambda: XA(0, 0)],
                }
                for i, _ in enumerate(qkv_units(2)):
                    for a in sched2.get(i, ()):
                        a()
                kv_begin(2, qkv_ps[2][2])
                kv_finish(1)
                rope_q_copy(2, qkv_ps[2])
                # chunk 3: attn(0) j2-3, attn(1) j0-2
                attn_begin(1)
                sched3 = {
                    1: [lambda: SA(0, 2)],
                    3: [lambda: XA(0, 1)],
                    4: [lambda: SA(0, 3)],
                    5: [lambda: vn_t(2)],
                    6: [lambda: XA(0, 2)],
                    7: [lambda: SA(1, 0)],
                    8: [lambda: vn_t(3)],
                    9: [lambda: XA(0, 3)],
                    10: [lambda: SA(1, 1)],
                    11: [lambda: rope_q_arith(2, 0)],
                    12: [lambda: XA(1, 0)],
                    13: [lambda: SA(1, 2)],
                    14: [lambda: rope_q_arith(2, 1)],
                    15: [lambda: XA(1, 1)],
                }
                for i, _ in enumerate(qkv_units(3)):
                    for a in sched3.get(i, ()):
                        a()
                kv_begin(3, qkv_ps[3][2])
                kv_finish(2)
                rope_q_copy(3, qkv_ps[3])

            # ---- Phase B ----
            def make_dense_units(pool, engines):
                def csb(c, h, stt):
                    if (c, h, stt) in all_csb:
                        return all_csb[(c, h, stt)]
                    return all_csb[(c, h)][:, stt * P:(stt + 1) * P]

                def dense_st(c, stt, tail):
                    ot = outp.tile([P, 4, FD], bf16, tag="ot", name=f"ot_{c}_{stt}")
                    for eo in range(4):
                        op = pool.tile([P, FD], f32, tag="o",
                                       name=f"o_{c}_{stt}_{eo}")
                        for h in range(2):
                            nc.tensor.matmul(
                                op,
                                lhsT=csb(c, h, stt),
                                rhs=wd_sb[:, h, eo * FD:(eo + 1) * FD],
                                start=(h == 0), stop=(h == 1),
                            )
                        if engines[eo] == "s":
                            nc.scalar.copy(ot[:, eo, :], op)
                        else:
                            nc.vector.tensor_copy(ot[:, eo, :], op)
                        if tail:
                            nc.sync.dma_start(out[c, stt, :, eo, :], ot[:, eo, :])
                        elif eo == 3:
                            nc.sync.dma_start(out[c, stt], ot)
                        yield

                def dense_units(c):
                    for stt in range(4):
                        yield from dense_st(c, stt, False)
                return dense_units, dense_st

            with tc.tile_pool(name="ps_sB", bufs=2, space="PSUM") as psB, \
                 tc.tile_pool(name="ps_o", bufs=2, space="PSUM") as ps_o:
                dense_units, dense_st = make_dense_units(ps_o, ["v", "v", "v", "s"])

                def SB(c, j):
                    pendB.append((c, attn_step(c, j, psB, split=False)))

                def XB(c):
                    cc, ent = pendB.pop(0)
                    assert cc == c
                    attn_ctx(c, ent)

                # B0: attn(1) j3..7 drain over dense(0)
                attn_tail(0, psB)
                rope_k(1)
                dq0 = dense_units(0)
                XA(1, 2)
                next(dq0); next(dq0)
                SB(1, 3)
                next(dq0); next(dq0)
                SB(1, 4)
                XB(1)
                next(dq0)
                vn_t(4)
                SB(1, 5)
                XB(1)
                next(dq0); next(dq0)
                vn_t(5)
                SB(1, 6)
                XB(1)
                next(dq0); next(dq0)
                vn_t(6)
                SB(1, 7)
                XB(1)
                next(dq0); next(dq0)
                vn_t(7)
                XB(1)
                next(dq0); next(dq0)
                attn_tail(1, psB)
                kv_finish(3)
                rope_q_arith(3, 0)
                rope_q_arith(3, 1)
                next(dq0); next(dq0); next(dq0)

                # B1: attn(2) j0..11 over dense(1)
                attn_begin(2)
                dq1 = dense_units(1)
                extras1 = {
                    0: [lambda: rope_k(2), lambda: vn_t(8)],
                    2: [lambda: vn_t(9)],
                    4: [lambda: vn_t(10)],
                    6: [lambda: vn_t(11)],
                }
                nd = 0
                for j in range(12):
                    for a in extras1.get(j, ()):
                        a()
                    SB(2, j)
                    want = (j + 1) * 16 // 12
                    while nd < want:
                        next(dq1)
                        nd += 1
                    if len(pendB) >= 3:
                        XB(2)
                while pendB:
                    XB(2)
                attn_tail(2, psB)

                # B2: attn(3) j0..11 over dense(2) (metered late: csb(2)
                # only lands shortly after T(2))
                attn_begin(3)
                dq2 = dense_units(2)
                extras2 = {
                    0: [lambda: rope_k(3)],
                    1: [lambda: vn_t(12)],
                    3: [lambda: vn_t(13)],
                    5: [lambda: vn_t(14)],
                    7: [lambda: vn_t(15)],
                }
                nd = 0
                for j in range(12):
                    for a in extras2.get(j, ()):
                        a()
                    SB(3, j)
                    want = 0 if j < 2 else min(12, (j - 1) * 16 // 10)
                    while nd < want:
                        next(dq2)
                        nd += 1
                    if len(pendB) >= 3:
                        XB(3)
                # pendB now holds j10, j11

                # B3: attn(3) j12..15 drain; per-st finalize; dense(3)
                dense_units3, dense_st3 = make_dense_units(ps_o, ["v", "s", "v", "s"])

                SB(3, 12); XB(3)                     # ctx j10
                next(dq2); next(dq2)
                SB(3, 13); XB(3)                     # ctx j11
                next(dq2); next(dq2)
                SB(3, 14); XB(3)                     # ctx j12
                fin_st(3, 0, ps_o, "o")
                SB(3, 15); XB(3)                     # ctx j13
                fin_st(3, 1, ps_o, "o")
                for _ in dense_st3(3, 0, False):
                    pass
                XB(3)                                # ctx j14
                fin_st(3, 2, ps_o, "o")
                for _ in dense_st3(3, 1, False):
                    pass
                XB(3)                                # ctx j15 (stop)
                fin_st(3, 3, ps_o, "o")
                for _ in dense_st3(3, 2, False):
                    pass
                for _ in dq2:
                    pass
                for _ in dense_st3(3, 3, True):
                    pass
    nc.compile()
    return nc


def make_in_maps(x, w_qkv, w_dense):
    x = np.asarray(x, np.float32).reshape(S, E)
    w_qkv = np.asarray(w_qkv, np.float32)
    w_dense = np.asarray(w_dense, np.float32)
    # x^T tiled to [sc, g, p, j, f] so each 512KB DMA block is contiguous
    xT = np.ascontiguousarray(x.T)
    xG = np.ascontiguousarray(
        xT.reshape(NG, 4, P, NSC, FD).transpose(3, 0, 2, 1, 4)
    ).astype(BF)
    consts = _host_constants()
    in_maps = []
    scale = np.float64(1.0 / np.sqrt(D))
    for d in range(NCORES):
        g = d // 2
        wq = w_qkv[2 * d * P:(2 * d + 2) * P] * scale
        if d % 2 == 0:
            wkv = w_qkv[H * D + g * P: H * D + (g + 1) * P]          # k head g
        else:
            wkv = w_qkv[H * D + KVH * D + g * P:
                        H * D + KVH * D + (g + 1) * P]               # v head g
        # f-blocks: [q0, q1, kv]
        wqkvT_d = np.ascontiguousarray(np.concatenate([wq, wkv], 0).T)
        wG_d = np.ascontiguousarray(
            wqkvT_d.reshape(NG, 4, P, FLOC).transpose(0, 2, 1, 3)
        ).astype(BF)
        wdT_d = w_dense[:, 2 * d * P:(2 * d + 2) * P].T  # [2P, S]
        wdG_d = np.ascontiguousarray(
            wdT_d.reshape(2, P, S).transpose(1, 0, 2)
        ).astype(BF)
        m = {"xG": xG, "wG": wG_d, "wdG": wdG_d}
        m.update(consts)
        in_maps.append(m)
    return in_maps


def kernel(x, w_qkv, w_dense):
    global LAST_RESULT, _BASS_CACHE
    from concourse.bass_utils import run_bass_kernel_spmd

    in_maps = make_in_maps(x, w_qkv, w_dense)
    if _BASS_CACHE is None:
        _BASS_CACHE = _build_bass()
    res = run_bass_kernel_spmd(_BASS_CACHE, in_maps, core_ids=list(range(NCORES)))
    LAST_RESULT = res
    # sum partials over cores; [c, st, p, eo, f] flattens straight to [s, e]
    acc = np.zeros((NSC, 4, P, 4, FD), np.float32)
    for r in res.results:
        acc += r["out"].astype(np.float32)
    return np.ascontiguousarray(acc.reshape(S, E)).reshape(B, S, E)


# revision 14
# speedup vs baseline: 1.2084x; 1.2084x over previous
"""Tensor-parallel MultiHeadAttention (QKV + RoPE + GQA causal SDPA + dense)
for 8 Trainium2 NeuronCores — bf16, hybrid kv-dedup edition (v6).

Sharding (TP as in TPMultiHeadAttention): core d owns query heads {2d, 2d+1}
and kv head d//2. kv handling is HYBRID:
  - chunks 0-1 (keys 0..1023): k,v projected locally on both cores of a
    pair (duplicated, baseline-style) so attention starts early;
  - chunks 2-3: core 2g projects only k_g, core 2g+1 only v_g (wKV input),
    and the pair exchanges raw kv chunks via DRAM bounce + pairwise
    AllGather. This halves the kv work for half the sequence (~7us of PE
    streaming per core).
The split is forced by a measured property of the NRT collective stack:
the first collective completes no earlier than ~65-80us after kernel
launch (comm-init + core launch skew) regardless of doorbell time, while
steady-state collectives take ~7-9us. Chunks 2-3's kv results are only
needed by attention j>=8 (~90us+), which the init floor meets; chunks 0-1
feed attention from ~30us and must be local. The two collectives are
serialized doorbell->readback->doorbell on the gpsimd queue (this also
re-aligns the cores), with cin DMAs on the scalar HWDGE ring and tail(1)'s
gpsimd broadcast emitted before readback(2) so it isn't blocked.

Schedule: phase A = baseline structure (chunk 0 plain; attn(0) inside
chunk 1; attn(1) spread over chunks 2-3 at 3-unit spacing; rope + XBAR v
transposes riding the qkv stream for jt 0-7). Phase B: attn(2) j0-7 over
dense(0)+dense(1); attn(2) j8-11 (cc-gated kr2) + attn(3) j0-7 over
dense(2); attn(3) j8-15 drain with per-128-column st-slice softmax
finalization of chunk 3 overlapped with dense(3). exp on ScalarE is the
phase-B serializer, so dense psum->SBUF copies split vector/scalar per eo.
All matmul operands bf16; PSUM fp32. Host sums the 8 bf16 partials.
"""

import numpy as np
import ml_dtypes

B, S, E = 1, 2048, 2048
H, KVH, D = 16, 4, 128
NCORES = 8
P = 128
FD = 512            # matmul moving free dim == one fp32 PSUM bank
NE = E // P         # 16 contraction tiles over the embedding dim
NG = 4              # eo-groups of 4
NSC = S // FD       # 4 sequence chunks
NST = S // P        # 16 sequence tiles
FLOC = 4 * P        # chunks 0-1: 2 q heads + k + v
ROPE_BASE = 10000.0
DIAG_START = (0, 128, 256, 384)
BF = ml_dtypes.bfloat16
GROUPS = [[0, 1], [2, 3], [4, 5], [6, 7]]

LAST_RESULT = None
_BASS_CACHE = None


def _rope_tables():
    inv = 1.0 / (ROPE_BASE ** (np.arange(0, D, 2, dtype=np.float64) / D))
    t = np.arange(S, dtype=np.float64)
    freqs = np.outer(t, inv)
    emb = np.concatenate([freqs, freqs], axis=-1)  # [S, D]
    return np.cos(emb), np.sin(emb)


def _host_constants():
    cos, sin = _rope_tables()
    cos_ds = np.ascontiguousarray(cos.T)  # [D, S]
    sin_ds = np.ascontiguousarray(sin.T)
    # sign-folded sin for the partition-shifted rotate-half:
    # tt[d] = qt[(d+64)%128] * sg[d],  sg = [-sin[:64]; +sin[64:]]
    sg = np.concatenate([-sin_ds[:64], sin_ds[64:]], axis=0)
    r_idx = np.arange(P)[:, None]
    c_idx = np.arange(P)[None, :]
    tri = (r_idx <= c_idx).astype(np.float64)
    return {
        "cosr": cos_ds.astype(BF),
        "sgsin": sg.astype(BF),
        "trim": tri.astype(BF),
        "ones": np.ones((P, 1), np.float64).astype(BF),
    }


def _build_bass():
    import concourse.mybir as mybir
    import concourse.tile as tile
    from concourse import bacc

    f32 = mybir.dt.float32
    bf16 = mybir.dt.bfloat16
    Exp = mybir.ActivationFunctionType.Exp

    nc = bacc.Bacc(None, target_bir_lowering=False, name="mha_tp8_v6")
    xG = nc.dram_tensor("xG", [NSC, NG, P, 4, FD], bf16, kind="ExternalInput")
    wG = nc.dram_tensor("wG", [NG, P, 4, FLOC], bf16, kind="ExternalInput")
    wKV = nc.dram_tensor("wKV", [NG, P, 4, P], bf16, kind="ExternalInput")
    wdG = nc.dram_tensor("wdG", [P, 2, S], bf16, kind="ExternalInput")
    cosr = nc.dram_tensor("cosr", [P, S], bf16, kind="ExternalInput")
    sgsin = nc.dram_tensor("sgsin", [P, S], bf16, kind="ExternalInput")
    trim = nc.dram_tensor("trim", [P, P], bf16, kind="ExternalInput")
    ones = nc.dram_tensor("ones", [P, 1], bf16, kind="ExternalInput")
    out = nc.dram_tensor("out", [NSC, 4, P, 4, FD], bf16, kind="ExternalOutput")

    with tile.TileContext(nc) as tc:
        with tc.tile_pool(name="const", bufs=1) as const, \
             tc.tile_pool(name="ps_ctx", bufs=2, space="PSUM") as ps_ctx, \
             tc.tile_pool(name="xs_p", bufs=5) as xpool, \
             tc.tile_pool(name="rtmp", bufs=3) as rtmp, \
             tc.tile_pool(name="kv_p", bufs=2) as kvp, \
             tc.tile_pool(name="pt_p", bufs=8) as ptp, \
             tc.tile_pool(name="acc_p", bufs=2) as accp, \
             tc.tile_pool(name="dn_p", bufs=2) as dnp, \
             tc.tile_pool(name="ctx_p", bufs=3) as ctxp, \
             tc.tile_pool(name="out_p", bufs=4) as outp, \
             tc.tile_pool(name="dram", bufs=2, space="DRAM") as dram:
            w_sb = const.tile([P, NE, FLOC], bf16, name="w_sb")
            wkv_sb = const.tile([P, NE, P], bf16, name="wkv_sb")
            cq = const.tile([P, S], bf16, name="cq")
            sg = const.tile([P, S], bf16, name="sg")
            mk = const.tile([P, P], bf16, name="mk")
            wd_sb = const.tile([P, 2, S], bf16, name="wd_sb")
            qr = const.tile([P, 2, S], bf16, name="qr")
            kr = const.tile([P, S], bf16, name="kr")
            vT = const.tile([P, S], bf16, name="vT")
            vn = const.tile([P, NST, P], bf16, name="vn")
            on = const.tile([P, 1], bf16, name="on")
            warm = const.tile([P, 8], bf16, name="warm")

            # ---- shared attention machinery (paired heads per j-tile) ----
            st_ = {}
            all_csb = {}

            def attn_begin(c):
                two = c >= 1
                ctxps = [
                    ps_ctx.tile([P, FD], f32, tag="ctx", name=f"ctx_{c}_{h}")
                    for h in range(2)
                ]
                accs = [
                    accp.tile([P, 2, FD], bf16, tag=f"acc{ch}", name=f"acc_{c}_{ch}")
                    for ch in range(2 if two else 1)
                ]
                st_[c] = (ctxps, accs, two)

            def attn_step(c, j, sdpool):
                ctxps, accs, two = st_[c]
                o = j - 4 * c
                so = DIAG_START[o] if o >= 0 else 0
                n = FD - so
                sd = sdpool.tile([P, 2, FD], f32, tag="sd", name=f"sd_{c}_{j}")
                for h in range(2):
                    nc.tensor.matmul(
                        sd[:, h, :n],
                        lhsT=kr[:, j * P:(j + 1) * P],
                        rhs=qr[:, h, c * FD + so:(c + 1) * FD],
                        start=True, stop=True,
                    )
                pt = ptp.tile([P, 2, FD], bf16, tag="pt", name=f"pt_{c}_{j}")
                nc.scalar.activation(pt[:, :, :n], sd[:, :, :n], Exp)
                if o >= 0:
                    for h in range(2):
                        nc.vector.tensor_mul(pt[:, h, :P], pt[:, h, :P], mk)
                acc = accs[j % 2 if two else 0]
                if j < (2 if two else 1):
                    nc.vector.tensor_copy(acc, pt)
                else:
                    nc.vector.tensor_add(acc[:, :, so:], acc[:, :, so:], pt[:, :, :n])
                return (j, pt, so, n)

            def attn_ctx(c, ent):
                ctxps, _, _ = st_[c]
                nj = 4 * c + 4
                j, pt, so, n = ent
                for h in range(2):
                    nc.tensor.matmul(
                        ctxps[h][:, so:],
                        lhsT=vn[:, j, :],
                        rhs=pt[:, h, :n],
                        start=(j == 0), stop=(j == nj - 1),
                    )

            def attn_tail(c, sdpool, ptag="sd"):
                ctxps, accs, two = st_[c]
                crs = []
                for h in range(2):
                    # unscaled PSUM->SBUF copy releases the ctx bank early
                    cr = ctxp.tile([P, FD], bf16, tag=f"cr{h}", name=f"cr_{c}_{h}")
                    nc.scalar.copy(cr, ctxps[h])
                    crs.append(cr)
                for h in range(2):
                    rp = sdpool.tile([1, FD], f32, tag=ptag, name=f"rp_{c}_{h}")
                    nc.tensor.matmul(rp, lhsT=on, rhs=accs[0][:, h, :],
                                     start=True, stop=not two)
                    if two:
                        nc.tensor.matmul(rp, lhsT=on, rhs=accs[1][:, h, :],
                                         start=False, stop=True)
                    rec = dnp.tile([1, FD], f32, tag=f"rec{h}", name=f"rec_{c}_{h}")
                    nc.vector.reciprocal_approx_fast(rec, rp)
                    rb = dnp.tile([P, FD], f32, tag=f"rb{h}", name=f"rb_{c}_{h}")
                    nc.gpsimd.partition_broadcast(rb, rec)
                    ct = ctxp.tile([P, FD], bf16, tag=f"ct{h}", name=f"csb_{c}_{h}")
                    nc.vector.tensor_mul(ct, crs[h], rb)
                    all_csb[(c, h)] = ct

            def fin_st(c, stt, rppool, ptag):
                # per-st-slice finalize for the last chunk: only needs
                # ctx/acc columns that are final after ctx matmul j=4c+stt
                ctxps, accs, two = st_[c]
                r = slice(stt * P, (stt + 1) * P)
                for h in range(2):
                    cr = ctxp.tile([P, P], bf16, tag=f"crs{h}",
                                   name=f"crs_{c}_{stt}_{h}")
                    nc.scalar.copy(cr, ctxps[h][:, r])
                    rp = rppool.tile([1, P], f32, tag=ptag,
                                     name=f"rps_{c}_{stt}_{h}")
                    nc.tensor.matmul(rp, lhsT=on, rhs=accs[0][:, h, r],
                                     start=True, stop=not two)
                    if two:
                        nc.tensor.matmul(rp, lhsT=on, rhs=accs[1][:, h, r],
                                         start=False, stop=True)
                    rec = dnp.tile([1, P], f32, tag=f"recs{h}",
                                   name=f"recs_{c}_{stt}_{h}")
                    nc.vector.reciprocal_approx_fast(rec, rp)
                    rb = dnp.tile([P, P], f32, tag=f"rbs{h}",
                                  name=f"rbs_{c}_{stt}_{h}")
                    nc.gpsimd.partition_broadcast(rb, rec)
                    ct = ctxp.tile([P, P], bf16, tag=f"cts{h}",
                                   name=f"cts_{c}_{stt}_{h}")
                    nc.vector.tensor_mul(ct, cr, rb)
                    all_csb[(c, h, stt)] = ct

            # ---- kv exchange (chunks 2,3 only) ----
            kv_parts = {}
            cc_bufs = {}

            def kv_begin(sc, kv_ps):
                kvsb = kvp.tile([P, FD], bf16, tag="kvsb", name=f"kvsb_{sc}")
                nc.scalar.copy(kvsb, kv_ps)
                cin = dram.tile([P, FD], bf16, tag="cin", name=f"cin_{sc}")
                cout = dram.tile([2, P, FD], bf16, tag="cout", name=f"cout_{sc}")
                nc.scalar.dma_start(cin, kvsb)
                nc.gpsimd.collective_compute(
                    "AllGather",
                    mybir.AluOpType.bypass,
                    replica_groups=GROUPS,
                    ins=[cin.opt()],
                    outs=[cout.opt()],
                )
                cc_bufs[sc] = cout

            def kv_finish(sc):
                ssl = slice(sc * FD, (sc + 1) * FD)
                cout = cc_bufs[sc]
                kraw = kvp.tile([P, FD], bf16, tag="kraw", name=f"kraw_{sc}")
                nc.gpsimd.dma_start(kraw, cout[0])
                nc.gpsimd.dma_start(vT[:, ssl], cout[1])
                kv_parts[sc] = kraw

            def rope_arith(dst, src, ssl):
                ts = rtmp.tile([P, FD], bf16, tag="ts", name="ts")
                nc.vector.tensor_copy(ts[0:64, :], src[64:128, :])
                nc.vector.tensor_copy(ts[64:128, :], src[0:64, :])
                tt = rtmp.tile([P, FD], bf16, tag="tt", name="tt")
                nc.vector.tensor_mul(tt, ts, sg[:, ssl])
                nc.vector.tensor_mul(dst, src, cq[:, ssl])
                nc.vector.tensor_add(dst, dst, tt)

            def rope_k(sc):
                ssl = slice(sc * FD, (sc + 1) * FD)
                rope_arith(kr[:, ssl], kv_parts[sc], ssl)

            def vn_t(jt):
                nc.sync.dma_start_transpose(vn[:, jt, :], vT[:, jt * P:(jt + 1) * P])

            pendB = []

            # ---- Phase A ----
            with tc.tile_pool(name="ps_qkv", bufs=1, space="PSUM") as pqkv, \
                 tc.tile_pool(name="ps_sA", bufs=1, space="PSUM") as psA:
                # tables ride the idle gpsimd ring
                nc.gpsimd.dma_start(mk, trim[:, :])
                nc.gpsimd.partition_broadcast(warm, mk[0:1, 0:8])
                nc.gpsimd.dma_start(cq, cosr[:, :])
                nc.gpsimd.dma_start(sg, sgsin[:, :])
                nc.gpsimd.dma_start(on, ones[:, :])

                qkv_ps = {}

                def qkv_units(sc):
                    """chunks 0-1: 4 matmuls/unit (full kv); 2-3: 3/unit."""
                    full = sc < 2
                    nf = 4 if full else 3
                    psums = [
                        pqkv.tile([P, FD], f32, tag=f"qkv{f}", name=f"ps_qkv{f}_{sc}")
                        for f in range(nf)
                    ]
                    qkv_ps[sc] = psums
                    for g in range(NG):
                        fine = sc == 0 and g == 0
                        if sc == 0 and not fine:
                            nc.scalar.dma_start(w_sb[:, 4 * g:4 * g + 4, :], wG[g])
                        if sc == 1:
                            # dedup-chunk kv weights + dense weights ride the
                            # scalar ring during chunk 1
                            nc.scalar.dma_start(wkv_sb[:, 4 * g:4 * g + 4, :], wKV[g])
                            if g == 0:
                                nc.scalar.dma_start(wd_sb, wdG[:, :, :])
                        xs = xpool.tile([P, 4, FD], bf16, tag="xs", name=f"xs_{sc}_{g}")
                        if not fine:
                            nc.sync.dma_start(xs, xG[sc, g])
                        for j in range(4):
                            if fine:
                                nc.scalar.dma_start(w_sb[:, j, :], wG[0, :, j, :])
                                nc.sync.dma_start(xs[:, j, :], xG[0, 0, :, j, :])
                            eo = 4 * g + j
                            if full:
                                for f in range(4):
                                    nc.tensor.matmul(
                                        psums[f],
                                        lhsT=w_sb[:, eo, f * P:(f + 1) * P],
                                        rhs=xs[:, j, :],
                                        start=(eo == 0), stop=(eo == NE - 1),
                                    )
                            else:
                                nc.tensor.matmul(
                                    psums[2],
                                    lhsT=wkv_sb[:, eo, :],
                                    rhs=xs[:, j, :],
                                    start=(eo == 0), stop=(eo == NE - 1),
                                )
                                for f in (0, 1):
                                    nc.tensor.matmul(
                                        psums[f],
                                        lhsT=w_sb[:, eo, f * P:(f + 1) * P],
                                        rhs=xs[:, j, :],
                                        start=(eo == 0), stop=(eo == NE - 1),
                                    )
                            if sc in (1, 2) and j == 3:
                                # v transposes for local chunks ride here
                                jt = 4 * (sc - 1) + g
                                nc.sync.dma_start_transpose(
                                    vn[:, jt, :], vT[:, jt * P:(jt + 1) * P]
                                )
                            yield

                def rope_vt(sc):
                    # chunks 0-1: rope q0,q1,k from psums + vT copy (local kv)
                    psums = qkv_ps[sc]
                    ssl = slice(sc * FD, (sc + 1) * FD)
                    for f in range(3):
                        dst = qr[:, f, ssl] if f < 2 else kr[:, ssl]
                        qt = rtmp.tile([P, FD], bf16, tag="qt", name=f"qt_{sc}_{f}")
                        nc.scalar.copy(qt, psums[f])
                        rope_arith(dst, qt, ssl)
                    nc.scalar.copy(vT[:, ssl], psums[3])

                def rope_q23(sc):
                    # chunks 2-3: rope q only (kv arrives via the exchange)
                    psums = qkv_ps[sc]
                    ssl = slice(sc * FD, (sc + 1) * FD)
                    for f in range(2):
                        qt = rtmp.tile([P, FD], bf16, tag="qt", name=f"qt_{sc}_{f}")
                        nc.scalar.copy(qt, psums[f])
                        rope_arith(qr[:, f, ssl], qt, ssl)

                # chunk 0: plain
                for _ in qkv_units(0):
                    pass
                rope_vt(0)
                # chunk 1 + attn(0): baseline schedule
                attn_begin(0)
                sched_s = {6: 0, 9: 1, 12: 2, 15: 3}
                sched_c = {8: 0, 11: 1, 14: 2}
                pend0 = {}
                for i, _ in enumerate(qkv_units(1)):
                    if i in sched_s:
                        pend0[sched_s[i]] = attn_step(0, sched_s[i], psA)
                    if i in sched_c:
                        attn_ctx(0, pend0.pop(sched_c[i]))
                attn_ctx(0, pend0.pop(3))
                attn_tail(0, psA)
                rope_vt(1)
                # chunks 2,3 (dedup units) + attn(1): 8 js over 32 units
                attn_begin(1)
                pend1 = {}
                base = 0
                for sc in (2, 3):
                    for i, _ in enumerate(qkv_units(sc)):
                        u = base + i
                        if u >= 3 and (u - 3) % 3 == 0 and (u - 3) // 3 < 8:
                            jx = (u - 3) // 3
                            pend1[jx] = attn_step(1, jx, psA)
                        if u >= 5 and (u - 5) % 3 == 0 and (u - 5) // 3 < 8:
                            attn_ctx(1, pend1.pop((u - 5) // 3))
                        if u == 27:
                            attn_tail(1, psA)
                    if sc == 2:
                        kv_begin(2, qkv_ps[2][2])
                        rope_q23(2)
                    base += 16
                kv_finish(2)
                kv_begin(3, qkv_ps[3][2])
                rope_q23(3)

            # ---- Phase B ----
            def make_dense_units(pool, engines):
                def csb(c, h, stt):
                    if (c, h, stt) in all_csb:
                        return all_csb[(c, h, stt)]
                    return all_csb[(c, h)][:, stt * P:(stt + 1) * P]

                def dense_st(c, stt, tail):
                    ot = outp.tile([P, 4, FD], bf16, tag="ot", name=f"ot_{c}_{stt}")
                    for eo in range(4):
                        op = pool.tile([P, FD], f32, tag="o",
                                       name=f"o_{c}_{stt}_{eo}")
                        for h in range(2):
                            nc.tensor.matmul(
                                op,
                                lhsT=csb(c, h, stt),
                                rhs=wd_sb[:, h, eo * FD:(eo + 1) * FD],
                                start=(h == 0), stop=(h == 1),
                            )
                        if engines[eo] == "s":
                            nc.scalar.copy(ot[:, eo, :], op)
                        else:
                            nc.vector.tensor_copy(ot[:, eo, :], op)
                        if tail:
                            nc.sync.dma_start(out[c, stt, :, eo, :], ot[:, eo, :])
                        elif eo == 3:
                            nc.sync.dma_start(out[c, stt], ot)
                        yield

                def dense_units(c):
                    for stt in range(4):
                        yield from dense_st(c, stt, False)
                return dense_units, dense_st

            with tc.tile_pool(name="ps_sB", bufs=2, space="PSUM") as psB, \
                 tc.tile_pool(name="ps_o", bufs=2, space="PSUM") as ps_o:
                dense_units, dense_st = make_dense_units(ps_o, ["v", "v", "v", "s"])

                def SB(c, j):
                    pendB.append((c, attn_step(c, j, psB)))

                def XB(c):
                    cc, ent = pendB.pop(0)
                    assert cc == c
                    attn_ctx(c, ent)

                # B0: attn(2) j0..7 over dense(0) + dense(1)
                attn_begin(2)
                dq0 = dense_units(0)
                dq1 = dense_units(1)
                nd = 0
                for j in range(8):
                    SB(2, j)
                    want = (j + 1) * 4
                    while nd < want:
                        next(dq0 if nd < 16 else dq1)
                        nd += 1
                    if len(pendB) >= 3:
                        XB(2)
                # j6, j7 ctx drain + exchange readbacks + rope
                XB(2)
                kv_finish(3)
                rope_k(2)
                vn_t(8)
                XB(2)
                vn_t(9)
                vn_t(10)
                vn_t(11)

                # B1: attn(2) j8..11 + T(2) + attn(3) j0..7 over dense(2)
                dq2 = dense_units(2)
                for j in range(8, 12):
                    SB(2, j)
                    if len(pendB) >= 2:
                        XB(2)
                XB(2)
                attn_tail(2, psB)
                attn_begin(3)
                extras3 = {
                    0: [lambda: rope_k(3)],
                    1: [lambda: vn_t(12)],
                    3: [lambda: vn_t(13)],
                    5: [lambda: vn_t(14)],
                    7: [lambda: vn_t(15)],
                }
                nd = 0
                for j in range(8):
                    for a in extras3.get(j, ()):
                        a()
                    SB(3, j)
                    want = 0 if j < 1 else j * 16 // 7
                    while nd < min(want, 12):
                        next(dq2)
                        nd += 1
                    if len(pendB) >= 3:
                        XB(3)
                # pendB: j5, j6, j7

                # B2/B3: attn(3) j8..15 drain; per-st finalize; dense(3)
                dense_units3, dense_st3 = make_dense_units(ps_o, ["v", "s", "v", "s"])
                SB(3, 8); XB(3)                      # ctx j5
                next(dq2)
                SB(3, 9); XB(3)                      # ctx j6
                next(dq2)
                SB(3, 10); XB(3)                     # ctx j7
                next(dq2)
                SB(3, 11); XB(3)                     # ctx j8
                next(dq2)
                SB(3, 12); XB(3)                     # ctx j9
                SB(3, 13); XB(3)                     # ctx j10
                SB(3, 14); XB(3)                     # ctx j11
                SB(3, 15); XB(3)                     # ctx j13
                fin_st(3, 0, ps_o, "o")              # needs ctx j12
                XB(3)                                # ctx j14
                fin_st(3, 1, ps_o, "o")              # needs ctx j13
                for _ in dense_st3(3, 0, False):
                    pass
                XB(3)                                # ctx j15 (stop)
                fin_st(3, 2, ps_o, "o")              # needs ctx j14
                for _ in dense_st3(3, 1, False):
                    pass
                fin_st(3, 3, ps_o, "o")              # needs ctx j15
                for _ in dense_st3(3, 2, False):
                    pass
                for _ in dq2:
                    pass
                for _ in dense_st3(3, 3, True):
                    pass
    nc.compile()
    return nc


def make_in_maps(x, w_qkv, w_dense):
    x = np.asarray(x, np.float32).reshape(S, E)
    w_qkv = np.asarray(w_qkv, np.float32)
    w_dense = np.asarray(w_dense, np.float32)
    # x^T tiled to [sc, g, p, j, f] so each 512KB DMA block is contiguous
    xT = np.ascontiguousarray(x.T)
    xG = np.ascontiguousarray(
        xT.reshape(NG, 4, P, NSC, FD).transpose(3, 0, 2, 1, 4)
    ).astype(BF)
    consts = _host_constants()
    in_maps = []
    scale = np.float64(1.0 / np.sqrt(D))
    for d in range(NCORES):
        g = d // 2
        wq = w_qkv[2 * d * P:(2 * d + 2) * P] * scale
        wk = w_qkv[H * D + g * P: H * D + (g + 1) * P]
        wv = w_qkv[H * D + KVH * D + g * P: H * D + KVH * D + (g + 1) * P]
        wqkvT_d = np.ascontiguousarray(np.concatenate([wq, wk, wv], 0).T)
        wG_d = np.ascontiguousarray(
            wqkvT_d.reshape(NG, 4, P, FLOC).transpose(0, 2, 1, 3)
        ).astype(BF)
        wkv_mine = wk if d % 2 == 0 else wv
        wKV_d = np.ascontiguousarray(
            np.ascontiguousarray(wkv_mine.T).reshape(NG, 4, P, P).transpose(0, 2, 1, 3)
        ).astype(BF)
        wdT_d = w_dense[:, 2 * d * P:(2 * d + 2) * P].T  # [2P, S]
        wdG_d = np.ascontiguousarray(
            wdT_d.reshape(2, P, S).transpose(1, 0, 2)
        ).astype(BF)
        m = {"xG": xG, "wG": wG_d, "wKV": wKV_d, "wdG": wdG_d}
        m.update(consts)
        in_maps.append(m)
    return in_maps


def kernel(x, w_qkv, w_dense):
    global LAST_RESULT, _BASS_CACHE
    from concourse.bass_utils import run_bass_kernel_spmd

    in_maps = make_in_maps(x, w_qkv, w_dense)
    if _BASS_CACHE is None:
        _BASS_CACHE = _build_bass()
    res = run_bass_kernel_spmd(_BASS_CACHE, in_maps, core_ids=list(range(NCORES)))
    LAST_RESULT = res
    # sum partials over cores; [c, st, p, eo, f] flattens straight to [s, e]
    acc = np.zeros((NSC, 4, P, 4, FD), np.float32)
    for r in res.results:
        acc += r["out"].astype(np.float32)
    return np.ascontiguousarray(acc.reshape(S, E)).reshape(B, S, E)


# revision 18
# speedup vs baseline: 1.2665x; 1.0481x over previous
"""Tensor-parallel MultiHeadAttention (QKV + RoPE + GQA causal SDPA + dense)
for 8 Trainium2 NeuronCores — bf16, hybrid kv-dedup edition (v6).

Sharding (TP as in TPMultiHeadAttention): core d owns query heads {2d, 2d+1}
and kv head d//2. kv handling is HYBRID:
  - chunks 0-1 (keys 0..1023): k,v projected locally on both cores of a
    pair (duplicated, baseline-style) so attention starts early;
  - chunks 2-3: core 2g projects only k_g, core 2g+1 only v_g (wKV input),
    and the pair exchanges raw kv chunks via DRAM bounce + pairwise
    AllGather. This halves the kv work for half the sequence (~7us of PE
    streaming per core).
The split is forced by a measured property of the NRT collective stack:
the first collective completes no earlier than ~65-80us after kernel
launch (comm-init + core launch skew) regardless of doorbell time, while
steady-state collectives take ~7-9us. Chunks 2-3's kv results are only
needed by attention j>=8 (~90us+), which the init floor meets; chunks 0-1
feed attention from ~30us and must be local. The two collectives are
serialized doorbell->readback->doorbell on the gpsimd queue (this also
re-aligns the cores), with cin DMAs on the scalar HWDGE ring and tail(1)'s
gpsimd broadcast emitted before readback(2) so it isn't blocked.

Schedule: phase A = baseline structure (chunk 0 plain; attn(0) inside
chunk 1; attn(1) spread over chunks 2-3 at 3-unit spacing; rope + XBAR v
transposes riding the qkv stream for jt 0-7). Phase B: attn(2) j0-7 over
dense(0)+dense(1); attn(2) j8-11 (cc-gated kr2) + attn(3) j0-7 over
dense(2); attn(3) j8-15 drain with per-128-column st-slice softmax
finalization of chunk 3 overlapped with dense(3). exp on ScalarE is the
phase-B serializer, so dense psum->SBUF copies split vector/scalar per eo.
All matmul operands bf16; PSUM fp32. Host sums the 8 bf16 partials.
"""

import numpy as np
import ml_dtypes

B, S, E = 1, 2048, 2048
H, KVH, D = 16, 4, 128
NCORES = 8
P = 128
FD = 512            # matmul moving free dim == one fp32 PSUM bank
NE = E // P         # 16 contraction tiles over the embedding dim
NG = 4              # eo-groups of 4
NSC = S // FD       # 4 sequence chunks
NST = S // P        # 16 sequence tiles
FLOC = 4 * P        # chunks 0-1: 2 q heads + k + v
ROPE_BASE = 10000.0
DIAG_START = (0, 128, 256, 384)
BF = ml_dtypes.bfloat16
GROUPS = [[0, 1], [2, 3], [4, 5], [6, 7]]

LAST_RESULT = None
_BASS_CACHE = None


def _rope_tables():
    inv = 1.0 / (ROPE_BASE ** (np.arange(0, D, 2, dtype=np.float64) / D))
    t = np.arange(S, dtype=np.float64)
    freqs = np.outer(t, inv)
    emb = np.concatenate([freqs, freqs], axis=-1)  # [S, D]
    return np.cos(emb), np.sin(emb)


def _host_constants():
    cos, sin = _rope_tables()
    cos_ds = np.ascontiguousarray(cos.T)  # [D, S]
    sin_ds = np.ascontiguousarray(sin.T)
    # sign-folded sin for the partition-shifted rotate-half:
    # tt[d] = qt[(d+64)%128] * sg[d],  sg = [-sin[:64]; +sin[64:]]
    sg = np.concatenate([-sin_ds[:64], sin_ds[64:]], axis=0)
    r_idx = np.arange(P)[:, None]
    c_idx = np.arange(P)[None, :]
    tri = (r_idx <= c_idx).astype(np.float64)
    return {
        "cosr": cos_ds.astype(BF),
        "sgsin": sg.astype(BF),
        "trim": tri.astype(BF),
        "ones": np.ones((P, 1), np.float64).astype(BF),
    }


def _build_bass():
    import concourse.mybir as mybir
    import concourse.tile as tile
    from concourse import bacc

    f32 = mybir.dt.float32
    bf16 = mybir.dt.bfloat16
    Exp = mybir.ActivationFunctionType.Exp

    nc = bacc.Bacc(None, target_bir_lowering=False, name="mha_tp8_v6")
    xG = nc.dram_tensor("xG", [NSC, NG, P, 4, FD], bf16, kind="ExternalInput")
    wG = nc.dram_tensor("wG", [NG, P, 4, FLOC], bf16, kind="ExternalInput")
    wKV = nc.dram_tensor("wKV", [NG, P, 4, P], bf16, kind="ExternalInput")
    wdG = nc.dram_tensor("wdG", [P, 2, S], bf16, kind="ExternalInput")
    cosr = nc.dram_tensor("cosr", [P, S], bf16, kind="ExternalInput")
    sgsin = nc.dram_tensor("sgsin", [P, S], bf16, kind="ExternalInput")
    trim = nc.dram_tensor("trim", [P, P], bf16, kind="ExternalInput")
    ones = nc.dram_tensor("ones", [P, 1], bf16, kind="ExternalInput")
    out = nc.dram_tensor("out", [NSC, 4, P, 4, FD], bf16, kind="ExternalOutput")

    with tile.TileContext(nc) as tc:
        with tc.tile_pool(name="const", bufs=1) as const, \
             tc.tile_pool(name="ps_ctx", bufs=2, space="PSUM") as ps_ctx, \
             tc.tile_pool(name="xs_p", bufs=5) as xpool, \
             tc.tile_pool(name="rtmp", bufs=3) as rtmp, \
             tc.tile_pool(name="kv_p", bufs=2) as kvp, \
             tc.tile_pool(name="pt_p", bufs=8) as ptp, \
             tc.tile_pool(name="acc_p", bufs=2) as accp, \
             tc.tile_pool(name="dn_p", bufs=2) as dnp, \
             tc.tile_pool(name="ctx_p", bufs=3) as ctxp, \
             tc.tile_pool(name="out_p", bufs=4) as outp, \
             tc.tile_pool(name="dram", bufs=2, space="DRAM") as dram:
            w_sb = const.tile([P, NE, FLOC], bf16, name="w_sb")
            wkv_sb = const.tile([P, NE, P], bf16, name="wkv_sb")
            cq = const.tile([P, S], bf16, name="cq")
            sg = const.tile([P, S], bf16, name="sg")
            mk = const.tile([P, P], bf16, name="mk")
            wd_sb = const.tile([P, 2, S], bf16, name="wd_sb")
            qr = const.tile([P, 2, S], bf16, name="qr")
            kr = const.tile([P, S], bf16, name="kr")
            vT = const.tile([P, S], bf16, name="vT")
            vn = const.tile([P, NST, P], bf16, name="vn")
            on = const.tile([P, 1], bf16, name="on")
            warm = const.tile([P, 8], bf16, name="warm")

            # ---- shared attention machinery (paired heads per j-tile) ----
            st_ = {}
            all_csb = {}

            def attn_begin(c):
                two = c >= 1
                ctxps = [
                    ps_ctx.tile([P, FD], f32, tag="ctx", name=f"ctx_{c}_{h}")
                    for h in range(2)
                ]
                accs = [
                    accp.tile([P, 2, FD], bf16, tag=f"acc{ch}", name=f"acc_{c}_{ch}")
                    for ch in range(2 if two else 1)
                ]
                st_[c] = (ctxps, accs, two)

            def attn_step(c, j, sdpool):
                ctxps, accs, two = st_[c]
                o = j - 4 * c
                so = DIAG_START[o] if o >= 0 else 0
                n = FD - so
                sd = sdpool.tile([P, 2, FD], f32, tag="sd", name=f"sd_{c}_{j}")
                for h in range(2):
                    nc.tensor.matmul(
                        sd[:, h, :n],
                        lhsT=kr[:, j * P:(j + 1) * P],
                        rhs=qr[:, h, c * FD + so:(c + 1) * FD],
                        start=True, stop=True,
                    )
                pt = ptp.tile([P, 2, FD], bf16, tag="pt", name=f"pt_{c}_{j}")
                nc.scalar.activation(pt[:, :, :n], sd[:, :, :n], Exp)
                if o >= 0:
                    for h in range(2):
                        nc.vector.tensor_mul(pt[:, h, :P], pt[:, h, :P], mk)
                acc = accs[j % 2 if two else 0]
                if j < (2 if two else 1):
                    nc.vector.tensor_copy(acc, pt)
                else:
                    nc.vector.tensor_add(acc[:, :, so:], acc[:, :, so:], pt[:, :, :n])
                return (j, pt, so, n)

            def attn_ctx(c, ent):
                ctxps, _, _ = st_[c]
                nj = 4 * c + 4
                j, pt, so, n = ent
                for h in range(2):
                    nc.tensor.matmul(
                        ctxps[h][:, so:],
                        lhsT=vn[:, j, :],
                        rhs=pt[:, h, :n],
                        start=(j == 0), stop=(j == nj - 1),
                    )

            def attn_tail(c, sdpool, ptag="sd"):
                ctxps, accs, two = st_[c]
                crs = []
                for h in range(2):
                    # unscaled PSUM->SBUF copy releases the ctx bank early
                    cr = ctxp.tile([P, FD], bf16, tag=f"cr{h}", name=f"cr_{c}_{h}")
                    nc.scalar.copy(cr, ctxps[h])
                    crs.append(cr)
                for h in range(2):
                    rp = sdpool.tile([1, FD], f32, tag=ptag, name=f"rp_{c}_{h}")
                    nc.tensor.matmul(rp, lhsT=on, rhs=accs[0][:, h, :],
                                     start=True, stop=not two)
                    if two:
                        nc.tensor.matmul(rp, lhsT=on, rhs=accs[1][:, h, :],
                                         start=False, stop=True)
                    rec = dnp.tile([1, FD], f32, tag=f"rec{h}", name=f"rec_{c}_{h}")
                    nc.vector.reciprocal_approx_fast(rec, rp)
                    rb = dnp.tile([P, FD], f32, tag=f"rb{h}", name=f"rb_{c}_{h}")
                    nc.gpsimd.partition_broadcast(rb, rec)
                    ct = ctxp.tile([P, FD], bf16, tag=f"ct{h}", name=f"csb_{c}_{h}")
                    nc.vector.tensor_mul(ct, crs[h], rb)
                    all_csb[(c, h)] = ct

            def fin_st(c, stt, rppool, ptag):
                # per-st-slice finalize for the last chunk: only needs
                # ctx/acc columns that are final after ctx matmul j=4c+stt
                ctxps, accs, two = st_[c]
                r = slice(stt * P, (stt + 1) * P)
                for h in range(2):
                    cr = ctxp.tile([P, P], bf16, tag=f"crs{h}",
                                   name=f"crs_{c}_{stt}_{h}")
                    nc.scalar.copy(cr, ctxps[h][:, r])
                    rp = rppool.tile([1, P], f32, tag=ptag,
                                     name=f"rps_{c}_{stt}_{h}")
                    nc.tensor.matmul(rp, lhsT=on, rhs=accs[0][:, h, r],
                                     start=True, stop=not two)
                    if two:
                        nc.tensor.matmul(rp, lhsT=on, rhs=accs[1][:, h, r],
                                         start=False, stop=True)
                    rec = dnp.tile([1, P], f32, tag=f"recs{h}",
                                   name=f"recs_{c}_{stt}_{h}")
                    nc.vector.reciprocal_approx_fast(rec, rp)
                    rb = dnp.tile([P, P], f32, tag=f"rbs{h}",
                                  name=f"rbs_{c}_{stt}_{h}")
                    nc.gpsimd.partition_broadcast(rb, rec)
                    ct = ctxp.tile([P, P], bf16, tag=f"cts{h}",
                                   name=f"cts_{c}_{stt}_{h}")
                    nc.vector.tensor_mul(ct, cr, rb)
                    all_csb[(c, h, stt)] = ct

            # ---- kv exchange (chunks 2,3 only) ----
            kv_parts = {}
            cc_bufs = {}

            def kv_begin(sc, kv_ps):
                kvsb = kvp.tile([P, FD], bf16, tag="kvsb", name=f"kvsb_{sc}")
                nc.scalar.copy(kvsb, kv_ps)
                cin = dram.tile([P, FD], bf16, tag="cin", name=f"cin_{sc}")
                cout = dram.tile([2, P, FD], bf16, tag="cout", name=f"cout_{sc}")
                nc.scalar.dma_start(cin, kvsb)
                nc.gpsimd.collective_compute(
                    "AllGather",
                    mybir.AluOpType.bypass,
                    replica_groups=GROUPS,
                    ins=[cin.opt()],
                    outs=[cout.opt()],
                )
                cc_bufs[sc] = cout

            def kv_finish(sc):
                ssl = slice(sc * FD, (sc + 1) * FD)
                cout = cc_bufs[sc]
                kraw = kvp.tile([P, FD], bf16, tag="kraw", name=f"kraw_{sc}")
                nc.gpsimd.dma_start(kraw, cout[0])
                nc.gpsimd.dma_start(vT[:, ssl], cout[1])
                kv_parts[sc] = kraw

            def rope_arith(dst, src, ssl):
                ts = rtmp.tile([P, FD], bf16, tag="ts", name="ts")
                nc.vector.tensor_copy(ts[0:64, :], src[64:128, :])
                nc.vector.tensor_copy(ts[64:128, :], src[0:64, :])
                tt = rtmp.tile([P, FD], bf16, tag="tt", name="tt")
                nc.vector.tensor_mul(tt, ts, sg[:, ssl])
                nc.vector.tensor_mul(dst, src, cq[:, ssl])
                nc.vector.tensor_add(dst, dst, tt)

            def rope_k(sc):
                ssl = slice(sc * FD, (sc + 1) * FD)
                rope_arith(kr[:, ssl], kv_parts[sc], ssl)

            def vn_t(jt):
                nc.sync.dma_start_transpose(vn[:, jt, :], vT[:, jt * P:(jt + 1) * P])

            pendB = []

            # ---- Phase A ----
            with tc.tile_pool(name="ps_qkv", bufs=1, space="PSUM") as pqkv, \
                 tc.tile_pool(name="ps_sA", bufs=1, space="PSUM") as psA:
                # tables ride the idle gpsimd ring
                nc.gpsimd.dma_start(mk, trim[:, :])
                nc.gpsimd.partition_broadcast(warm, mk[0:1, 0:8])
                nc.gpsimd.dma_start(cq, cosr[:, :])
                nc.gpsimd.dma_start(sg, sgsin[:, :])
                nc.gpsimd.dma_start(on, ones[:, :])
                # wd rides the gpsimd ring too (trigger now, lands ~45us,
                # needed ~64us): on the scalar ring its completion shared a
                # semaphore group with the cin DMAs and gated the collective
                # doorbell ~20us late
                nc.gpsimd.dma_start(wd_sb, wdG[:, :, :])

                qkv_ps = {}

                def qkv_units(sc):
                    """chunks 0-1: 4 matmuls/unit (full kv); 2-3: 3/unit."""
                    full = sc < 2
                    nf = 4 if full else 3
                    psums = [
                        pqkv.tile([P, FD], f32, tag=f"qkv{f}", name=f"ps_qkv{f}_{sc}")
                        for f in range(nf)
                    ]
                    qkv_ps[sc] = psums
                    for g in range(NG):
                        fine = sc == 0 and g == 0
                        if sc == 0 and not fine:
                            nc.scalar.dma_start(w_sb[:, 4 * g:4 * g + 4, :], wG[g])
                        if sc == 1:
                            # dedup-chunk kv weights ride the scalar ring
                            nc.scalar.dma_start(wkv_sb[:, 4 * g:4 * g + 4, :], wKV[g])
                        xs = xpool.tile([P, 4, FD], bf16, tag="xs", name=f"xs_{sc}_{g}")
                        if not fine:
                            nc.sync.dma_start(xs, xG[sc, g])
                        for j in range(4):
                            if fine:
                                nc.scalar.dma_start(w_sb[:, j, :], wG[0, :, j, :])
                                nc.sync.dma_start(xs[:, j, :], xG[0, 0, :, j, :])
                            eo = 4 * g + j
                            if full:
                                for f in range(4):
                                    nc.tensor.matmul(
                                        psums[f],
                                        lhsT=w_sb[:, eo, f * P:(f + 1) * P],
                                        rhs=xs[:, j, :],
                                        start=(eo == 0), stop=(eo == NE - 1),
                                    )
                            else:
                                nc.tensor.matmul(
                                    psums[2],
                                    lhsT=wkv_sb[:, eo, :],
                                    rhs=xs[:, j, :],
                                    start=(eo == 0), stop=(eo == NE - 1),
                                )
                                for f in (0, 1):
                                    nc.tensor.matmul(
                                        psums[f],
                                        lhsT=w_sb[:, eo, f * P:(f + 1) * P],
                                        rhs=xs[:, j, :],
                                        start=(eo == 0), stop=(eo == NE - 1),
                                    )
                            if sc in (1, 2) and j == 3:
                                # v transposes for local chunks ride here
                                jt = 4 * (sc - 1) + g
                                nc.sync.dma_start_transpose(
                                    vn[:, jt, :], vT[:, jt * P:(jt + 1) * P]
                                )
                            yield

                def rope_vt(sc):
                    # chunks 0-1: rope q0,q1,k from psums + vT copy (local kv)
                    psums = qkv_ps[sc]
                    ssl = slice(sc * FD, (sc + 1) * FD)
                    for f in range(3):
                        dst = qr[:, f, ssl] if f < 2 else kr[:, ssl]
                        qt = rtmp.tile([P, FD], bf16, tag="qt", name=f"qt_{sc}_{f}")
                        nc.scalar.copy(qt, psums[f])
                        rope_arith(dst, qt, ssl)
                    nc.scalar.copy(vT[:, ssl], psums[3])

                def rope_q23(sc):
                    # chunks 2-3: rope q only (kv arrives via the exchange)
                    psums = qkv_ps[sc]
                    ssl = slice(sc * FD, (sc + 1) * FD)
                    for f in range(2):
                        qt = rtmp.tile([P, FD], bf16, tag="qt", name=f"qt_{sc}_{f}")
                        nc.scalar.copy(qt, psums[f])
                        rope_arith(qr[:, f, ssl], qt, ssl)

                # chunk 0: plain
                for _ in qkv_units(0):
                    pass
                rope_vt(0)
                # chunk 1 + attn(0): baseline schedule
                attn_begin(0)
                sched_s = {6: 0, 9: 1, 12: 2, 15: 3}
                sched_c = {8: 0, 11: 1, 14: 2}
                pend0 = {}
                for i, _ in enumerate(qkv_units(1)):
                    if i in sched_s:
                        pend0[sched_s[i]] = attn_step(0, sched_s[i], psA)
                    if i in sched_c:
                        attn_ctx(0, pend0.pop(sched_c[i]))
                attn_ctx(0, pend0.pop(3))
                attn_tail(0, psA)
                rope_vt(1)
                # chunks 2,3 (dedup units) + attn(1): 8 js over 32 units
                attn_begin(1)
                pend1 = {}
                base = 0
                for sc in (2, 3):
                    for i, _ in enumerate(qkv_units(sc)):
                        u = base + i
                        if u >= 3 and (u - 3) % 3 == 0 and (u - 3) // 3 < 8:
                            jx = (u - 3) // 3
                            pend1[jx] = attn_step(1, jx, psA)
                        if u >= 5 and (u - 5) % 3 == 0 and (u - 5) // 3 < 8:
                            attn_ctx(1, pend1.pop((u - 5) // 3))
                        if u == 27:
                            attn_tail(1, psA)
                    if sc == 2:
                        kv_begin(2, qkv_ps[2][2])
                        rope_q23(2)
                    base += 16
                kv_finish(2)
                kv_begin(3, qkv_ps[3][2])
                rope_q23(3)

            # ---- Phase B ----
            def make_dense_units(pool, engines):
                def csb(c, h, stt):
                    if (c, h, stt) in all_csb:
                        return all_csb[(c, h, stt)]
                    return all_csb[(c, h)][:, stt * P:(stt + 1) * P]

                def dense_st(c, stt, tail):
                    ot = outp.tile([P, 4, FD], bf16, tag="ot", name=f"ot_{c}_{stt}")
                    for eo in range(4):
                        op = pool.tile([P, FD], f32, tag="o",
                                       name=f"o_{c}_{stt}_{eo}")
                        for h in range(2):
                            nc.tensor.matmul(
                                op,
                                lhsT=csb(c, h, stt),
                                rhs=wd_sb[:, h, eo * FD:(eo + 1) * FD],
                                start=(h == 0), stop=(h == 1),
                            )
                        if engines[eo] == "s":
                            nc.scalar.copy(ot[:, eo, :], op)
                        else:
                            nc.vector.tensor_copy(ot[:, eo, :], op)
                        if tail:
                            # final tile: drain over both HWDGE rings
                            eng = nc.sync if eo % 2 == 0 else nc.scalar
                            eng.dma_start(out[c, stt, :, eo, :], ot[:, eo, :])
                        elif eo == 3:
                            nc.sync.dma_start(out[c, stt], ot)
                        yield

                def dense_units(c):
                    for stt in range(4):
                        yield from dense_st(c, stt, False)
                return dense_units, dense_st

            with tc.tile_pool(name="ps_sB", bufs=2, space="PSUM") as psB, \
                 tc.tile_pool(name="ps_o", bufs=2, space="PSUM") as ps_o:
                dense_units, dense_st = make_dense_units(ps_o, ["v", "v", "v", "s"])

                def SB(c, j):
                    pendB.append((c, attn_step(c, j, psB)))

                def XB(c):
                    cc, ent = pendB.pop(0)
                    assert cc == c
                    attn_ctx(c, ent)

                # B0: attn(2) j0..7 over dense(0) + dense(1)
                attn_begin(2)
                dq0 = dense_units(0)
                dq1 = dense_units(1)
                nd = 0
                for j in range(8):
                    SB(2, j)
                    want = (j + 1) * 4
                    while nd < want:
                        next(dq0 if nd < 16 else dq1)
                        nd += 1
                    if len(pendB) >= 3:
                        XB(2)
                # j6, j7 ctx drain + exchange readbacks + rope
                XB(2)
                kv_finish(3)
                rope_k(2)
                vn_t(8)
                XB(2)
                vn_t(9)
                vn_t(10)
                vn_t(11)

                # B1: attn(2) j8..11 + T(2) + attn(3) j0..7 over dense(2)
                dq2 = dense_units(2)
                for j in range(8, 12):
                    SB(2, j)
                    if len(pendB) >= 2:
                        XB(2)
                XB(2)
                attn_tail(2, psB)
                attn_begin(3)
                extras3 = {
                    0: [lambda: rope_k(3)],
                    1: [lambda: vn_t(12)],
                    3: [lambda: vn_t(13)],
                    5: [lambda: vn_t(14)],
                    7: [lambda: vn_t(15)],
                }
                nd = 0
                for j in range(8):
                    for a in extras3.get(j, ()):
                        a()
                    SB(3, j)
                    want = 0 if j < 1 else j * 16 // 7
                    while nd < min(want, 12):
                        next(dq2)
                        nd += 1
                    if len(pendB) >= 3:
                        XB(3)
                # pendB: j5, j6, j7

                # B2/B3: attn(3) j8..15 drain; per-st finalize; dense(3).
                # dense copies all-scalar here: the DVE must stay clear for
                # the per-st finalize chain that gates each dense_st
                dense_units3, dense_st3 = make_dense_units(ps_o, ["s", "s", "s", "s"])
                SB(3, 8); XB(3)                      # ctx j5
                next(dq2)
                SB(3, 9); XB(3)                      # ctx j6
                next(dq2)
                SB(3, 10); XB(3)                     # ctx j7
                next(dq2)
                SB(3, 11); XB(3)                     # ctx j8
                next(dq2)
                SB(3, 12); XB(3)                     # ctx j9
                SB(3, 13); XB(3)                     # ctx j10
                SB(3, 14); XB(3)                     # ctx j11
                SB(3, 15); XB(3)                     # ctx j13
                fin_st(3, 0, ps_o, "o")              # needs ctx j12
                XB(3)                                # ctx j14
                fin_st(3, 1, ps_o, "o")              # needs ctx j13
                for _ in dense_st3(3, 0, False):
                    pass
                XB(3)                                # ctx j15 (stop)
                fin_st(3, 2, ps_o, "o")              # needs ctx j14
                for _ in dense_st3(3, 1, False):
                    pass
                fin_st(3, 3, ps_o, "o")              # needs ctx j15
                for _ in dense_st3(3, 2, False):
                    pass
                for _ in dq2:
                    pass
                for _ in dense_st3(3, 3, True):
                    pass
    nc.compile()
    return nc


def make_in_maps(x, w_qkv, w_dense):
    x = np.asarray(x, np.float32).reshape(S, E)
    w_qkv = np.asarray(w_qkv, np.float32)
    w_dense = np.asarray(w_dense, np.float32)
    # x^T tiled to [sc, g, p, j, f] so each 512KB DMA block is contiguous
    xT = np.ascontiguousarray(x.T)
    xG = np.ascontiguousarray(
        xT.reshape(NG, 4, P, NSC, FD).transpose(3, 0, 2, 1, 4)
    ).astype(BF)
    consts = _host_constants()
    in_maps = []
    scale = np.float64(1.0 / np.sqrt(D))
    for d in range(NCORES):
        g = d // 2
        wq = w_qkv[2 * d * P:(2 * d + 2) * P] * scale
        wk = w_qkv[H * D + g * P: H * D + (g + 1) * P]
        wv = w_qkv[H * D + KVH * D + g * P: H * D + KVH * D + (g + 1) * P]
        wqkvT_d = np.ascontiguousarray(np.concatenate([wq, wk, wv], 0).T)
        wG_d = np.ascontiguousarray(
            wqkvT_d.reshape(NG, 4, P, FLOC).transpose(0, 2, 1, 3)
        ).astype(BF)
        wkv_mine = wk if d % 2 == 0 else wv
        wKV_d = np.ascontiguousarray(
            np.ascontiguousarray(wkv_mine.T).reshape(NG, 4, P, P).transpose(0, 2, 1, 3)
        ).astype(BF)
        wdT_d = w_dense[:, 2 * d * P:(2 * d + 2) * P].T  # [2P, S]
        wdG_d = np.ascontiguousarray(
            wdT_d.reshape(2, P, S).transpose(1, 0, 2)
        ).astype(BF)
        m = {"xG": xG, "wG": wG_d, "wKV": wKV_d, "wdG": wdG_d}
        m.update(consts)
        in_maps.append(m)
    return in_maps


def kernel(x, w_qkv, w_dense):
    global LAST_RESULT, _BASS_CACHE
    from concourse.bass_utils import run_bass_kernel_spmd

    in_maps = make_in_maps(x, w_qkv, w_dense)
    if _BASS_CACHE is None:
        _BASS_CACHE = _build_bass()
    res = run_bass_kernel_spmd(_BASS_CACHE, in_maps, core_ids=list(range(NCORES)))
    LAST_RESULT = res
    # sum partials over cores; [c, st, p, eo, f] flattens straight to [s, e]
    acc = np.zeros((NSC, 4, P, 4, FD), np.float32)
    for r in res.results:
        acc += r["out"].astype(np.float32)
    return np.ascontiguousarray(acc.reshape(S, E)).reshape(B, S, E)


# revision 21
# speedup vs baseline: 1.3177x; 1.0405x over previous
"""Tensor-parallel MultiHeadAttention (QKV + RoPE + GQA causal SDPA + dense)
for 8 Trainium2 NeuronCores — bf16, software-pipelined edition.

Sharding (TP as in TPMultiHeadAttention): core d owns query heads {2d, 2d+1}
and the single kv head d//2 (kv heads replicated across core pairs), plus the
matching 256 columns of the dense projection. Each core produces a full-shape
partial output; the all-reduce is a host-side sum over the 8 bf16 partials.

All matmul operands are bf16 (same PE column rate as float32r but half the
LDWEIGHTS time, half the DMA bytes, 2x DVE rate); PSUM accumulation is fp32.

Schedule highlights (from perfetto analysis of earlier revisions):
  - exp on ScalarE (616ns/tile) is slower than a score+ctx matmul pair
    (432ns), so the two heads' score tiles share one 2-bank PSUM tile and a
    single exp instruction covers both ([128, 2, n]).
  - attention for chunks 0/1 (and chunk 2's first two producer steps)
    trickles INSIDE the QKV phase, where 54us of projection work hides the
    exp latency; chunks 2 and 3 are software-pipelined with ctx lagging
    scores by 3 slots, with dense units of earlier chunks metered in as PE
    filler (exp at ~1.1us/j outpaces the 864ns of score+ctx matmuls).
  - v^T -> v transposes use the DMA XBAR (16-bit only), freeing PSUM banks
    and the PE; rotate_half is two partition-shifted DVE copies.
  - softmax denominators: bf16 DVE accumulation chains, column-summed by a
    ones-vector matmul (borrowing a PSUM bank from the sd/o rings), DVE
    reciprocal, gpsimd partition broadcast.  The ctx PSUM banks are released
    early by an unscaled ScalarE copy; the 1/denominator multiply happens in
    SBUF off the ring-critical path.  gpsimd's ucode library load (~7us) is
    prefetched by a dummy broadcast at kernel start, and the rope/mask
    tables ride the otherwise-idle gpsimd DMA ring.
"""

import numpy as np
import ml_dtypes

B, S, E = 1, 2048, 2048
H, KVH, D = 16, 4, 128
NCORES = 8
P = 128
FD = 512            # matmul moving free dim == one fp32 PSUM bank
NE = E // P         # 16 contraction tiles over the embedding dim
NG = 4              # eo-groups of 4 (one 512KB DMA each)
NSC = S // FD       # 4 sequence chunks
NST = S // P        # 16 sequence tiles
FLOC = 4 * P        # local fused qkv rows per core (2 q heads + k + v)
ROPE_BASE = 10000.0
# causally visible query sub-range start for diagonal sk tile o
DIAG_START = (0, 128, 256, 384)
BF = ml_dtypes.bfloat16

LAST_RESULT = None
_BASS_CACHE = None


def _rope_tables():
    inv = 1.0 / (ROPE_BASE ** (np.arange(0, D, 2, dtype=np.float64) / D))
    t = np.arange(S, dtype=np.float64)
    freqs = np.outer(t, inv)
    emb = np.concatenate([freqs, freqs], axis=-1)  # [S, D]
    return np.cos(emb), np.sin(emb)


def _host_constants():
    cos, sin = _rope_tables()
    cos_ds = np.ascontiguousarray(cos.T)  # [D, S]
    sin_ds = np.ascontiguousarray(sin.T)
    # sign-folded sin for the partition-shifted rotate-half:
    # tt[d] = qt[(d+64)%128] * sg[d],  sg = [-sin[:64]; +sin[64:]]
    sg = np.concatenate([-sin_ds[:64], sin_ds[64:]], axis=0)
    r_idx = np.arange(P)[:, None]
    c_idx = np.arange(P)[None, :]
    tri = (r_idx <= c_idx).astype(np.float64)
    return {
        "cosr": cos_ds.astype(BF),
        "sgsin": sg.astype(BF),
        "trim": tri.astype(BF),
        "ones": np.ones((P, 1), np.float64).astype(BF),
    }


def _build_bass():
    import concourse.mybir as mybir
    import concourse.tile as tile
    from concourse import bacc

    f32 = mybir.dt.float32
    bf16 = mybir.dt.bfloat16
    Exp = mybir.ActivationFunctionType.Exp

    nc = bacc.Bacc(None, target_bir_lowering=False, name="mha_tp8_v3")
    xG = nc.dram_tensor("xG", [NSC, NG, P, 4, FD], bf16, kind="ExternalInput")
    wG = nc.dram_tensor("wG", [NG, P, 4, FLOC], bf16, kind="ExternalInput")
    wdG = nc.dram_tensor("wdG", [P, 2, S], bf16, kind="ExternalInput")
    cosr = nc.dram_tensor("cosr", [P, S], bf16, kind="ExternalInput")
    sgsin = nc.dram_tensor("sgsin", [P, S], bf16, kind="ExternalInput")
    trim = nc.dram_tensor("trim", [P, P], bf16, kind="ExternalInput")
    ones = nc.dram_tensor("ones", [P, 1], bf16, kind="ExternalInput")
    out = nc.dram_tensor("out", [NSC, 4, P, 4, FD], bf16, kind="ExternalOutput")

    with tile.TileContext(nc) as tc:
        with tc.tile_pool(name="const", bufs=1) as const, \
             tc.tile_pool(name="ps_ctx", bufs=2, space="PSUM") as ps_ctx, \
             tc.tile_pool(name="xs_p", bufs=5) as xpool, \
             tc.tile_pool(name="rtmp", bufs=3) as rtmp, \
             tc.tile_pool(name="pt_p", bufs=8) as ptp, \
             tc.tile_pool(name="acc_p", bufs=2) as accp, \
             tc.tile_pool(name="dn_p", bufs=2) as dnp, \
             tc.tile_pool(name="ctx_p", bufs=3) as ctxp, \
             tc.tile_pool(name="out_p", bufs=3) as outp:
            w_sb = const.tile([P, NE, FLOC], bf16, name="w_sb")
            cq = const.tile([P, S], bf16, name="cq")
            sg = const.tile([P, S], bf16, name="sg")
            mk = const.tile([P, P], bf16, name="mk")
            wd_sb = const.tile([P, 2, S], bf16, name="wd_sb")
            qr = const.tile([P, 2, S], bf16, name="qr")
            kr = const.tile([P, S], bf16, name="kr")
            vT = const.tile([P, S], bf16, name="vT")
            vn = const.tile([P, NST, P], bf16, name="vn")
            on = const.tile([P, 1], bf16, name="on")
            warm = const.tile([P, 8], bf16, name="warm")

            # ---- shared attention machinery (paired heads per j-tile) ----
            st_ = {}          # per-chunk attention state
            all_csb = {}

            def attn_begin(c):
                two = c >= 1
                ctxps = [
                    ps_ctx.tile([P, FD], f32, tag="ctx", name=f"ctx_{c}_{h}")
                    for h in range(2)
                ]
                accs = [
                    accp.tile([P, 2, FD], bf16, tag=f"acc{ch}", name=f"acc_{c}_{ch}")
                    for ch in range(2 if two else 1)
                ]
                st_[c] = (ctxps, accs, two)

            def attn_step(c, j, sdpool):
                ctxps, accs, two = st_[c]
                o = j - 4 * c
                so = DIAG_START[o] if o >= 0 else 0
                n = FD - so
                sd = sdpool.tile([P, 2, FD], f32, tag="sd", name=f"sd_{c}_{j}")
                for h in range(2):
                    nc.tensor.matmul(
                        sd[:, h, :n],
                        lhsT=kr[:, j * P:(j + 1) * P],
                        rhs=qr[:, h, c * FD + so:(c + 1) * FD],
                        start=True, stop=True,
                    )
                pt = ptp.tile([P, 2, FD], bf16, tag="pt", name=f"pt_{c}_{j}")
                nc.scalar.activation(pt[:, :, :n], sd[:, :, :n], Exp)
                if o >= 0:
                    for h in range(2):
                        nc.vector.tensor_mul(pt[:, h, :P], pt[:, h, :P], mk)
                acc = accs[j % 2 if two else 0]
                if j < (2 if two else 1):
                    nc.vector.tensor_copy(acc, pt)
                else:
                    nc.vector.tensor_add(acc[:, :, so:], acc[:, :, so:], pt[:, :, :n])
                return (j, pt, so, n)

            def attn_ctx(c, ent):
                ctxps, _, _ = st_[c]
                nj = 4 * c + 4
                j, pt, so, n = ent
                for h in range(2):
                    nc.tensor.matmul(
                        ctxps[h][:, so:],
                        lhsT=vn[:, j, :],
                        rhs=pt[:, h, :n],
                        start=(j == 0), stop=(j == nj - 1),
                    )

            def attn_tail(c, sdpool, ptag="sd"):
                ctxps, accs, two = st_[c]
                crs = []
                for h in range(2):
                    # unscaled PSUM->SBUF copy releases the ctx bank ~0.7us
                    # after the last ctx matmul (the scale chain is ~2.5us
                    # and would otherwise stall the next chunk's ctx ring)
                    cr = ctxp.tile([P, FD], bf16, tag=f"cr{h}", name=f"cr_{c}_{h}")
                    nc.scalar.copy(cr, ctxps[h])
                    crs.append(cr)
                for h in range(2):
                    # column sums via a ones-vector matmul into the sd ring
                    rp = sdpool.tile([1, FD], f32, tag=ptag, name=f"rp_{c}_{h}")
                    nc.tensor.matmul(rp, lhsT=on, rhs=accs[0][:, h, :],
                                     start=True, stop=not two)
                    if two:
                        nc.tensor.matmul(rp, lhsT=on, rhs=accs[1][:, h, :],
                                         start=False, stop=True)
                    rec = dnp.tile([1, FD], f32, tag=f"rec{h}", name=f"rec_{c}_{h}")
                    nc.vector.reciprocal_approx_fast(rec, rp)
                    rb = dnp.tile([P, FD], f32, tag=f"rb{h}", name=f"rb_{c}_{h}")
                    nc.gpsimd.partition_broadcast(rb, rec)
                    ct = ctxp.tile([P, FD], bf16, tag=f"ct{h}", name=f"csb_{c}_{h}")
                    nc.vector.tensor_mul(ct, crs[h], rb)
                    all_csb[(c, h)] = ct

            def fin_st(c, stt, rppool, ptag):
                # per-128-column softmax finalize for the last chunk: slice
                # stt only needs ctx/acc columns that are final after ctx
                # matmul j=4c+stt, so dense(3, stt) can start while the
                # attention drain is still running
                ctxps, accs, two = st_[c]
                r = slice(stt * P, (stt + 1) * P)
                for h in range(2):
                    cr = ctxp.tile([P, P], bf16, tag=f"crs{h}",
                                   name=f"crs_{c}_{stt}_{h}")
                    nc.scalar.copy(cr, ctxps[h][:, r])
                    rp = rppool.tile([1, P], f32, tag=ptag,
                                     name=f"rps_{c}_{stt}_{h}")
                    nc.tensor.matmul(rp, lhsT=on, rhs=accs[0][:, h, r],
                                     start=True, stop=not two)
                    if two:
                        nc.tensor.matmul(rp, lhsT=on, rhs=accs[1][:, h, r],
                                         start=False, stop=True)
                    rec = dnp.tile([1, P], f32, tag=f"recs{h}",
                                   name=f"recs_{c}_{stt}_{h}")
                    nc.vector.reciprocal_approx_fast(rec, rp)
                    rb = dnp.tile([P, P], f32, tag=f"rbs{h}",
                                  name=f"rbs_{c}_{stt}_{h}")
                    nc.gpsimd.partition_broadcast(rb, rec)
                    ct = ctxp.tile([P, P], bf16, tag=f"cts{h}",
                                   name=f"cts_{c}_{stt}_{h}")
                    nc.vector.tensor_mul(ct, cr, rb)
                    all_csb[(c, h, stt)] = ct

            # ---- Phase A: QKV + RoPE + v-transpose, attn(0/1) trickled ----
            with tc.tile_pool(name="ps_qkv", bufs=1, space="PSUM") as pqkv, \
                 tc.tile_pool(name="ps_sA", bufs=1, space="PSUM") as psA:
                # tables ride the idle gpsimd ring so the sync ring's first
                # w/x transfers keep most of the DMA bandwidth
                nc.gpsimd.dma_start(mk, trim[:, :])
                nc.gpsimd.partition_broadcast(warm, mk[0:1, 0:8])
                nc.gpsimd.dma_start(cq, cosr[:, :])
                nc.gpsimd.dma_start(sg, sgsin[:, :])
                nc.gpsimd.dma_start(on, ones[:, :])

                qkv_ps = {}

                def qkv_units(sc):
                    """Yields after each (g, j) group of 4 matmuls (~864ns PE)."""
                    psums = [
                        pqkv.tile([P, FD], f32, tag=f"qkv{f}", name=f"ps_qkv{f}_{sc}")
                        for f in range(4)
                    ]
                    qkv_ps[sc] = psums
                    for g in range(NG):
                        fine = sc == 0 and g == 0
                        if sc == 0 and not fine:
                            nc.scalar.dma_start(w_sb[:, 4 * g:4 * g + 4, :], wG[g])
                        xs = xpool.tile([P, 4, FD], bf16, tag="xs", name=f"xs_{sc}_{g}")
                        if not fine:
                            nc.sync.dma_start(xs, xG[sc, g])
                        for j in range(4):
                            if fine:
                                # 128KB pieces, weights on the idle scalar
                                # ring, so the first matmuls aren't starved
                                nc.scalar.dma_start(w_sb[:, j, :], wG[0, :, j, :])
                                nc.sync.dma_start(xs[:, j, :], xG[0, 0, :, j, :])
                            eo = 4 * g + j
                            for f in range(4):
                                nc.tensor.matmul(
                                    psums[f],
                                    lhsT=w_sb[:, eo, f * P:(f + 1) * P],
                                    rhs=xs[:, j, :],
                                    start=(eo == 0),
                                    stop=(eo == NE - 1),
                                )
                            if sc >= 1 and j == 3:
                                # previous chunk's v-transpose rides here so a
                                # blocked XBAR issue can't head-of-line block
                                # the x-tile DMAs on the sync ring
                                jt = 4 * (sc - 1) + g
                                nc.sync.dma_start_transpose(
                                    vn[:, jt, :], vT[:, jt * P:(jt + 1) * P]
                                )
                            yield
                    if sc == 3:
                        nc.sync.dma_start(wd_sb, wdG[:, :, :])

                def rope_vt(sc):
                    psums = qkv_ps[sc]
                    ssl = slice(sc * FD, (sc + 1) * FD)
                    for f in range(3):
                        dst = qr[:, f, ssl] if f < 2 else kr[:, ssl]
                        qt = rtmp.tile([P, FD], bf16, tag="qt", name=f"qt_{sc}_{f}")
                        nc.scalar.copy(qt, psums[f])
                        # rotate_half = two partition-shifted DVE copies
                        ts = rtmp.tile([P, FD], bf16, tag="ts", name=f"ts_{sc}_{f}")
                        nc.vector.tensor_copy(ts[0:64, :], qt[64:128, :])
                        nc.vector.tensor_copy(ts[64:128, :], qt[0:64, :])
                        tt = rtmp.tile([P, FD], bf16, tag="tt", name=f"tt_{sc}_{f}")
                        nc.vector.tensor_mul(tt, ts, sg[:, ssl])
                        nc.vector.tensor_mul(dst, qt, cq[:, ssl])
                        nc.vector.tensor_add(dst, dst, tt)
                    nc.scalar.copy(vT[:, ssl], psums[3])

                # chunk 0: plain
                for _ in qkv_units(0):
                    pass
                rope_vt(0)
                # chunk 1 + attn(0): 4 js at units 6,9,12,15; ctx 2 units later
                attn_begin(0)
                sched_s = {6: 0, 9: 1, 12: 2, 15: 3}
                sched_c = {8: 0, 11: 1, 14: 2}
                pend0 = {}
                for i, _ in enumerate(qkv_units(1)):
                    if i in sched_s:
                        pend0[sched_s[i]] = attn_step(0, sched_s[i], psA)
                    if i in sched_c:
                        attn_ctx(0, pend0.pop(sched_c[i]))
                attn_ctx(0, pend0.pop(3))
                attn_tail(0, psA)
                rope_vt(1)
                # chunks 2,3 + attn(1): 8 js over 32 units, spacing 4
                attn_begin(1)
                pend1 = {}
                pend2 = []
                base = 0
                for sc in (2, 3):
                    for i, _ in enumerate(qkv_units(sc)):
                        u = base + i
                        if u >= 3 and (u - 3) % 3 == 0 and (u - 3) // 3 < 8:
                            jx = (u - 3) // 3
                            pend1[jx] = attn_step(1, jx, psA)
                        if u >= 5 and (u - 5) % 3 == 0 and (u - 5) // 3 < 8:
                            attn_ctx(1, pend1.pop((u - 5) // 3))
                        if u == 27:
                            # attn(1) fully drained by u=26; start attn(2)'s
                            # producer side under the remaining QKV stream
                            attn_tail(1, psA)
                            attn_begin(2)
                            pend2.append(attn_step(2, 0, psA))
                        if u == 30:
                            pend2.append(attn_step(2, 1, psA))
                    if sc == 2:
                        rope_vt(2)
                    base += 16
                rope_vt(3)
                for jt in range(12, 16):
                    nc.sync.dma_start_transpose(vn[:, jt, :], vT[:, jt * P:(jt + 1) * P])

            # ---- Phase B: attn(2/3) pipelined + dense ----
            def make_dense_units(pool):
                def dense_units(c, tail):
                    for st in range(4):
                        ot = outp.tile([P, 4, FD], bf16, tag="ot", name=f"ot_{c}_{st}")
                        for eo in range(4):
                            op = pool.tile([P, FD], f32, tag="o", name=f"o_{c}_{st}_{eo}")
                            for h in range(2):
                                nc.tensor.matmul(
                                    op,
                                    lhsT=all_csb[(c, h)][:, st * P:(st + 1) * P],
                                    rhs=wd_sb[:, h, eo * FD:(eo + 1) * FD],
                                    start=(h == 0), stop=(h == 1),
                                )
                            if (eo % 2) if tail else (eo == 3):
                                nc.scalar.copy(ot[:, eo, :], op)
                            else:
                                nc.vector.tensor_copy(ot[:, eo, :], op)
                            if tail and c == 3 and st == 3:
                                nc.sync.dma_start(out[c, st, :, eo, :], ot[:, eo, :])
                            elif eo == 3:
                                nc.sync.dma_start(out[c, st], ot)
                            yield
                return dense_units

            with tc.tile_pool(name="ps_sB", bufs=2, space="PSUM") as psB, \
                 tc.tile_pool(name="ps_o", bufs=2, space="PSUM") as ps_o:
                dense_units = make_dense_units(ps_o)

                def emit_attn_B(c, dq, pend=None, jstart=0, leave=0):
                    nj = 4 * c + 4
                    if pend is None:
                        attn_begin(c)
                        pend = []
                    nd = 0
                    for j in range(jstart, nj):
                        pend.append(attn_step(c, j, psB))
                        if dq is not None:
                            want = (j + 1 - jstart) * 16 // (nj - jstart)
                            while nd < want:
                                next(dq)
                                nd += 1
                        if len(pend) >= 3:
                            attn_ctx(c, pend.pop(0))
                    while len(pend) > leave:
                        attn_ctx(c, pend.pop(0))
                    if dq is not None:
                        for _ in dq:
                            pass
                    return pend

                emit_attn_B(2, dense_units(0, False), pend=pend2, jstart=2)
                attn_tail(2, psB)
                # keep ctx j14/j15 pending so the per-st finalize can
                # interleave with the drain (ctx j12/j13 emit in-loop)
                pend3 = emit_attn_B(3, dense_units(1, False), leave=2)
            with tc.tile_pool(name="ps_d", bufs=4, space="PSUM") as ps_d:
                dense_tail = make_dense_units(ps_d)

                def dense3_st(stt, last):
                    # chunk-3 dense per st-slice, copies all-scalar so the
                    # DVE stays clear for the finalize chains gating them
                    ot = outp.tile([P, 4, FD], bf16, tag="ot", name=f"o3t_{stt}")
                    for eo in range(4):
                        op = ps_d.tile([P, FD], f32, tag="o", name=f"o3_{stt}_{eo}")
                        for h in range(2):
                            nc.tensor.matmul(
                                op,
                                lhsT=all_csb[(3, h, stt)],
                                rhs=wd_sb[:, h, eo * FD:(eo + 1) * FD],
                                start=(h == 0), stop=(h == 1),
                            )
                        nc.scalar.copy(ot[:, eo, :], op)
                        if last:
                            # final tile: drain over both HWDGE rings
                            eng = nc.sync if eo % 2 == 0 else nc.scalar
                            eng.dma_start(out[3, stt, :, eo, :], ot[:, eo, :])
                        elif eo == 3:
                            nc.sync.dma_start(out[3, stt], ot)

                # dense(2) streams on the PE while chunk 3's per-st softmax
                # finalize chains drain; dense(3, st) starts as soon as its
                # slice is final
                dq2 = dense_tail(2, True)
                for _ in range(4):
                    next(dq2)
                fin_st(3, 0, ps_d, "o")              # needs ctx j12 (in-loop)
                for _ in range(2):
                    next(dq2)
                fin_st(3, 1, ps_d, "o")              # needs ctx j13 (in-loop)
                for _ in range(2):
                    next(dq2)
                attn_ctx(3, pend3.pop(0))            # ctx j14
                fin_st(3, 2, ps_d, "o")
                dense3_st(0, False)
                for _ in range(2):
                    next(dq2)
                dense3_st(1, False)
                attn_ctx(3, pend3.pop(0))            # ctx j15 (stop)
                fin_st(3, 3, ps_d, "o")
                dense3_st(2, False)
                for _ in dq2:
                    pass
                dense3_st(3, True)
    nc.compile()
    return nc


def make_in_maps(x, w_qkv, w_dense):
    x = np.asarray(x, np.float32).reshape(S, E)
    w_qkv = np.asarray(w_qkv, np.float32)
    w_dense = np.asarray(w_dense, np.float32)
    # x^T tiled to [sc, g, p, j, f] so each 512KB DMA block is contiguous
    xT = np.ascontiguousarray(x.T)
    xG = np.ascontiguousarray(
        xT.reshape(NG, 4, P, NSC, FD).transpose(3, 0, 2, 1, 4)
    ).astype(BF)
    consts = _host_constants()
    in_maps = []
    scale = np.float64(1.0 / np.sqrt(D))
    for d in range(NCORES):
        g = d // 2
        wq = w_qkv[2 * d * P:(2 * d + 2) * P] * scale
        wk = w_qkv[H * D + g * P: H * D + (g + 1) * P]
        wv = w_qkv[H * D + KVH * D + g * P: H * D + KVH * D + (g + 1) * P]
        wqkvT_d = np.ascontiguousarray(np.concatenate([wq, wk, wv], 0).T)
        wG_d = np.ascontiguousarray(
            wqkvT_d.reshape(NG, 4, P, FLOC).transpose(0, 2, 1, 3)
        ).astype(BF)
        wdT_d = w_dense[:, 2 * d * P:(2 * d + 2) * P].T  # [2P, S]
        wdG_d = np.ascontiguousarray(
            wdT_d.reshape(2, P, S).transpose(1, 0, 2)
        ).astype(BF)
        m = {"xG": xG, "wG": wG_d, "wdG": wdG_d}
        m.update(consts)
        in_maps.append(m)
    return in_maps


def kernel(x, w_qkv, w_dense):
    global LAST_RESULT, _BASS_CACHE
    from concourse.bass_utils import run_bass_kernel_spmd

    in_maps = make_in_maps(x, w_qkv, w_dense)
    if _BASS_CACHE is None:
        _BASS_CACHE = _build_bass()
    res = run_bass_kernel_spmd(_BASS_CACHE, in_maps, core_ids=list(range(NCORES)))
    LAST_RESULT = res
    # sum partials over cores; [c, st, p, eo, f] flattens straight to [s, e]
    acc = np.zeros((NSC, 4, P, 4, FD), np.float32)
    for r in res.results:
        acc += r["out"].astype(np.float32)
    return np.ascontiguousarray(acc.reshape(S, E)).reshape(B, S, E)



# revision 22
# speedup vs baseline: 1.3389x; 1.0161x over previous
"""Tensor-parallel MultiHeadAttention (QKV + RoPE + GQA causal SDPA + dense)
for 8 Trainium2 NeuronCores — bf16, software-pipelined edition.

Sharding (TP as in TPMultiHeadAttention): core d owns query heads {2d, 2d+1}
and the single kv head d//2 (kv heads replicated across core pairs), plus the
matching 256 columns of the dense projection. Each core produces a full-shape
partial output; the all-reduce is a host-side sum over the 8 bf16 partials.

All matmul operands are bf16 (same PE column rate as float32r but half the
LDWEIGHTS time, half the DMA bytes, 2x DVE rate); PSUM accumulation is fp32.

Schedule highlights (from perfetto analysis of earlier revisions):
  - exp on ScalarE (616ns/tile) is slower than a score+ctx matmul pair
    (432ns), so the two heads' score tiles share one 2-bank PSUM tile and a
    single exp instruction covers both ([128, 2, n]).
  - attention for chunks 0/1 (and chunk 2's first two producer steps)
    trickles INSIDE the QKV phase, where 54us of projection work hides the
    exp latency; chunks 2 and 3 are software-pipelined with ctx lagging
    scores by 3 slots, with dense units of earlier chunks metered in as PE
    filler (exp at ~1.1us/j outpaces the 864ns of score+ctx matmuls).
  - v^T -> v transposes use the DMA XBAR (16-bit only), freeing PSUM banks
    and the PE; rotate_half is two partition-shifted DVE copies.
  - softmax denominators: bf16 DVE accumulation chains, column-summed by a
    ones-vector matmul (borrowing a PSUM bank from the sd/o rings), DVE
    reciprocal, gpsimd partition broadcast.  The ctx PSUM banks are released
    early by an unscaled ScalarE copy; the 1/denominator multiply happens in
    SBUF off the ring-critical path.  gpsimd's ucode library load (~7us) is
    prefetched by a dummy broadcast at kernel start, and the rope/mask
    tables ride the otherwise-idle gpsimd DMA ring.
"""

import numpy as np
import ml_dtypes

B, S, E = 1, 2048, 2048
H, KVH, D = 16, 4, 128
NCORES = 8
P = 128
FD = 512            # matmul moving free dim == one fp32 PSUM bank
NE = E // P         # 16 contraction tiles over the embedding dim
NG = 4              # eo-groups of 4 (one 512KB DMA each)
NSC = S // FD       # 4 sequence chunks
NST = S // P        # 16 sequence tiles
FLOC = 4 * P        # local fused qkv rows per core (2 q heads + k + v)
ROPE_BASE = 10000.0
# causally visible query sub-range start for diagonal sk tile o
DIAG_START = (0, 128, 256, 384)
BF = ml_dtypes.bfloat16

LAST_RESULT = None
_BASS_CACHE = None


def _rope_tables():
    inv = 1.0 / (ROPE_BASE ** (np.arange(0, D, 2, dtype=np.float64) / D))
    t = np.arange(S, dtype=np.float64)
    freqs = np.outer(t, inv)
    emb = np.concatenate([freqs, freqs], axis=-1)  # [S, D]
    return np.cos(emb), np.sin(emb)


def _host_constants():
    cos, sin = _rope_tables()
    cos_ds = np.ascontiguousarray(cos.T)  # [D, S]
    sin_ds = np.ascontiguousarray(sin.T)
    # sign-folded sin for the partition-shifted rotate-half:
    # tt[d] = qt[(d+64)%128] * sg[d],  sg = [-sin[:64]; +sin[64:]]
    sg = np.concatenate([-sin_ds[:64], sin_ds[64:]], axis=0)
    r_idx = np.arange(P)[:, None]
    c_idx = np.arange(P)[None, :]
    tri = (r_idx <= c_idx).astype(np.float64)
    return {
        "cosr": cos_ds.astype(BF),
        "sgsin": sg.astype(BF),
        "trim": tri.astype(BF),
        "ones": np.ones((P, 1), np.float64).astype(BF),
    }


def _build_bass():
    import concourse.mybir as mybir
    import concourse.tile as tile
    from concourse import bacc

    f32 = mybir.dt.float32
    bf16 = mybir.dt.bfloat16
    Exp = mybir.ActivationFunctionType.Exp

    nc = bacc.Bacc(None, target_bir_lowering=False, name="mha_tp8_v3")
    xG = nc.dram_tensor("xG", [NSC, NG, P, 4, FD], bf16, kind="ExternalInput")
    wG = nc.dram_tensor("wG", [NG, P, 4, FLOC], bf16, kind="ExternalInput")
    wdG = nc.dram_tensor("wdG", [P, 2, S], bf16, kind="ExternalInput")
    cosr = nc.dram_tensor("cosr", [P, S], bf16, kind="ExternalInput")
    sgsin = nc.dram_tensor("sgsin", [P, S], bf16, kind="ExternalInput")
    trim = nc.dram_tensor("trim", [P, P], bf16, kind="ExternalInput")
    ones = nc.dram_tensor("ones", [P, 1], bf16, kind="ExternalInput")
    out = nc.dram_tensor("out", [NSC, 4, P, 4, FD], bf16, kind="ExternalOutput")

    with tile.TileContext(nc) as tc:
        with tc.tile_pool(name="const", bufs=1) as const, \
             tc.tile_pool(name="ps_ctx", bufs=2, space="PSUM") as ps_ctx, \
             tc.tile_pool(name="xs_p", bufs=5) as xpool, \
             tc.tile_pool(name="rtmp", bufs=3) as rtmp, \
             tc.tile_pool(name="pt_p", bufs=8) as ptp, \
             tc.tile_pool(name="acc_p", bufs=2) as accp, \
             tc.tile_pool(name="dn_p", bufs=2) as dnp, \
             tc.tile_pool(name="ctx_p", bufs=3) as ctxp, \
             tc.tile_pool(name="out_p", bufs=3) as outp:
            w_sb = const.tile([P, NE, FLOC], bf16, name="w_sb")
            cq = const.tile([P, S], bf16, name="cq")
            sg = const.tile([P, S], bf16, name="sg")
            mk = const.tile([P, P], bf16, name="mk")
            wd_sb = const.tile([P, 2, S], bf16, name="wd_sb")
            qr = const.tile([P, 2, S], bf16, name="qr")
            kr = const.tile([P, S], bf16, name="kr")
            vT = const.tile([P, S], bf16, name="vT")
            vn = const.tile([P, NST, P], bf16, name="vn")
            on = const.tile([P, 1], bf16, name="on")
            warm = const.tile([P, 8], bf16, name="warm")

            # ---- shared attention machinery (paired heads per j-tile) ----
            st_ = {}          # per-chunk attention state
            all_csb = {}

            def attn_begin(c):
                two = c >= 1
                ctxps = [
                    ps_ctx.tile([P, FD], f32, tag="ctx", name=f"ctx_{c}_{h}")
                    for h in range(2)
                ]
                accs = [
                    accp.tile([P, 2, FD], bf16, tag=f"acc{ch}", name=f"acc_{c}_{ch}")
                    for ch in range(2 if two else 1)
                ]
                st_[c] = (ctxps, accs, two)

            def attn_step(c, j, sdpool):
                ctxps, accs, two = st_[c]
                o = j - 4 * c
                so = DIAG_START[o] if o >= 0 else 0
                n = FD - so
                sd = sdpool.tile([P, 2, FD], f32, tag="sd", name=f"sd_{c}_{j}")
                for h in range(2):
                    nc.tensor.matmul(
                        sd[:, h, :n],
                        lhsT=kr[:, j * P:(j + 1) * P],
                        rhs=qr[:, h, c * FD + so:(c + 1) * FD],
                        start=True, stop=True,
                    )
                pt = ptp.tile([P, 2, FD], bf16, tag="pt", name=f"pt_{c}_{j}")
                nc.scalar.activation(pt[:, :, :n], sd[:, :, :n], Exp)
                if o >= 0:
                    for h in range(2):
                        nc.vector.tensor_mul(pt[:, h, :P], pt[:, h, :P], mk)
                acc = accs[j % 2 if two else 0]
                if j < (2 if two else 1):
                    nc.vector.tensor_copy(acc, pt)
                else:
                    nc.vector.tensor_add(acc[:, :, so:], acc[:, :, so:], pt[:, :, :n])
                return (j, pt, so, n)

            def attn_ctx(c, ent):
                ctxps, _, _ = st_[c]
                nj = 4 * c + 4
                j, pt, so, n = ent
                for h in range(2):
                    nc.tensor.matmul(
                        ctxps[h][:, so:],
                        lhsT=vn[:, j, :],
                        rhs=pt[:, h, :n],
                        start=(j == 0), stop=(j == nj - 1),
                    )

            def attn_tail(c, sdpool, ptag="sd"):
                ctxps, accs, two = st_[c]
                crs = []
                for h in range(2):
                    # unscaled PSUM->SBUF copy releases the ctx bank ~0.7us
                    # after the last ctx matmul (the scale chain is ~2.5us
                    # and would otherwise stall the next chunk's ctx ring)
                    cr = ctxp.tile([P, FD], bf16, tag=f"cr{h}", name=f"cr_{c}_{h}")
                    nc.scalar.copy(cr, ctxps[h])
                    crs.append(cr)
                for h in range(2):
                    # column sums via a ones-vector matmul into the sd ring
                    rp = sdpool.tile([1, FD], f32, tag=ptag, name=f"rp_{c}_{h}")
                    nc.tensor.matmul(rp, lhsT=on, rhs=accs[0][:, h, :],
                                     start=True, stop=not two)
                    if two:
                        nc.tensor.matmul(rp, lhsT=on, rhs=accs[1][:, h, :],
                                         start=False, stop=True)
                    rec = dnp.tile([1, FD], f32, tag=f"rec{h}", name=f"rec_{c}_{h}")
                    nc.vector.reciprocal_approx_fast(rec, rp)
                    rb = dnp.tile([P, FD], f32, tag=f"rb{h}", name=f"rb_{c}_{h}")
                    nc.gpsimd.partition_broadcast(rb, rec)
                    ct = ctxp.tile([P, FD], bf16, tag=f"ct{h}", name=f"csb_{c}_{h}")
                    nc.vector.tensor_mul(ct, crs[h], rb)
                    all_csb[(c, h)] = ct

            def fin_st(c, stt, rppool, ptag):
                # per-128-column softmax finalize for the last chunk: slice
                # stt only needs ctx/acc columns that are final after ctx
                # matmul j=4c+stt, so dense(3, stt) can start while the
                # attention drain is still running
                ctxps, accs, two = st_[c]
                r = slice(stt * P, (stt + 1) * P)
                for h in range(2):
                    cr = ctxp.tile([P, P], bf16, tag=f"crs{h}",
                                   name=f"crs_{c}_{stt}_{h}")
                    nc.scalar.copy(cr, ctxps[h][:, r])
                    rp = rppool.tile([1, P], f32, tag=ptag,
                                     name=f"rps_{c}_{stt}_{h}")
                    nc.tensor.matmul(rp, lhsT=on, rhs=accs[0][:, h, r],
                                     start=True, stop=not two)
                    if two:
                        nc.tensor.matmul(rp, lhsT=on, rhs=accs[1][:, h, r],
                                         start=False, stop=True)
                    rec = dnp.tile([1, P], f32, tag=f"recs{h}",
                                   name=f"recs_{c}_{stt}_{h}")
                    nc.vector.reciprocal_approx_fast(rec, rp)
                    rb = dnp.tile([P, P], f32, tag=f"rbs{h}",
                                  name=f"rbs_{c}_{stt}_{h}")
                    nc.gpsimd.partition_broadcast(rb, rec)
                    ct = ctxp.tile([P, P], bf16, tag=f"cts{h}",
                                   name=f"cts_{c}_{stt}_{h}")
                    nc.vector.tensor_mul(ct, cr, rb)
                    all_csb[(c, h, stt)] = ct

            # ---- Phase A: QKV + RoPE + v-transpose, attn(0/1) trickled ----
            with tc.tile_pool(name="ps_qkv", bufs=1, space="PSUM") as pqkv, \
                 tc.tile_pool(name="ps_sA", bufs=1, space="PSUM") as psA:
                # tables ride the idle gpsimd ring so the sync ring's first
                # w/x transfers keep most of the DMA bandwidth
                nc.gpsimd.dma_start(mk, trim[:, :])
                nc.gpsimd.partition_broadcast(warm, mk[0:1, 0:8])
                nc.gpsimd.dma_start(cq, cosr[:, :])
                nc.gpsimd.dma_start(sg, sgsin[:, :])
                nc.gpsimd.dma_start(on, ones[:, :])

                qkv_ps = {}

                def qkv_units(sc):
                    """Yields after each (g, j) group of 4 matmuls (~864ns PE)."""
                    psums = [
                        pqkv.tile([P, FD], f32, tag=f"qkv{f}", name=f"ps_qkv{f}_{sc}")
                        for f in range(4)
                    ]
                    qkv_ps[sc] = psums
                    for g in range(NG):
                        fine = sc == 0 and g == 0
                        if sc == 0 and not fine:
                            nc.scalar.dma_start(w_sb[:, 4 * g:4 * g + 4, :], wG[g])
                        xs = xpool.tile([P, 4, FD], bf16, tag="xs", name=f"xs_{sc}_{g}")
                        if not fine:
                            nc.sync.dma_start(xs, xG[sc, g])
                        for j in range(4):
                            if fine:
                                # 128KB pieces, weights on the idle scalar
                                # ring, so the first matmuls aren't starved
                                nc.scalar.dma_start(w_sb[:, j, :], wG[0, :, j, :])
                                nc.sync.dma_start(xs[:, j, :], xG[0, 0, :, j, :])
                            eo = 4 * g + j
                            for f in range(4):
                                nc.tensor.matmul(
                                    psums[f],
                                    lhsT=w_sb[:, eo, f * P:(f + 1) * P],
                                    rhs=xs[:, j, :],
                                    start=(eo == 0),
                                    stop=(eo == NE - 1),
                                )
                            if sc >= 1 and j == 3:
                                # previous chunk's v-transpose rides here so a
                                # blocked XBAR issue can't head-of-line block
                                # the x-tile DMAs on the sync ring
                                jt = 4 * (sc - 1) + g
                                nc.sync.dma_start_transpose(
                                    vn[:, jt, :], vT[:, jt * P:(jt + 1) * P]
                                )
                            yield
                    if sc == 3:
                        nc.sync.dma_start(wd_sb, wdG[:, :, :])

                def rope_vt(sc):
                    psums = qkv_ps[sc]
                    ssl = slice(sc * FD, (sc + 1) * FD)
                    for f in range(3):
                        dst = qr[:, f, ssl] if f < 2 else kr[:, ssl]
                        qt = rtmp.tile([P, FD], bf16, tag="qt", name=f"qt_{sc}_{f}")
                        nc.scalar.copy(qt, psums[f])
                        # rotate_half = two partition-shifted DVE copies
                        ts = rtmp.tile([P, FD], bf16, tag="ts", name=f"ts_{sc}_{f}")
                        nc.vector.tensor_copy(ts[0:64, :], qt[64:128, :])
                        nc.vector.tensor_copy(ts[64:128, :], qt[0:64, :])
                        tt = rtmp.tile([P, FD], bf16, tag="tt", name=f"tt_{sc}_{f}")
                        nc.vector.tensor_mul(tt, ts, sg[:, ssl])
                        nc.vector.tensor_mul(dst, qt, cq[:, ssl])
                        nc.vector.tensor_add(dst, dst, tt)
                    nc.scalar.copy(vT[:, ssl], psums[3])

                # chunk 0: plain
                for _ in qkv_units(0):
                    pass
                rope_vt(0)
                # chunk 1 + attn(0): 4 js at units 6,9,12,15; ctx 2 units later
                attn_begin(0)
                sched_s = {6: 0, 9: 1, 12: 2, 15: 3}
                sched_c = {8: 0, 11: 1, 14: 2}
                pend0 = {}
                for i, _ in enumerate(qkv_units(1)):
                    if i in sched_s:
                        pend0[sched_s[i]] = attn_step(0, sched_s[i], psA)
                    if i in sched_c:
                        attn_ctx(0, pend0.pop(sched_c[i]))
                attn_ctx(0, pend0.pop(3))
                attn_tail(0, psA)
                rope_vt(1)
                # chunks 2,3 + attn(1): 8 js over 32 units, spacing 4
                attn_begin(1)
                pend1 = {}
                pend2 = []
                base = 0
                for sc in (2, 3):
                    for i, _ in enumerate(qkv_units(sc)):
                        u = base + i
                        if u >= 3 and (u - 3) % 3 == 0 and (u - 3) // 3 < 8:
                            jx = (u - 3) // 3
                            pend1[jx] = attn_step(1, jx, psA)
                        if u >= 5 and (u - 5) % 3 == 0 and (u - 5) // 3 < 8:
                            attn_ctx(1, pend1.pop((u - 5) // 3))
                        if u == 27:
                            # attn(1) fully drained by u=26; start attn(2)'s
                            # producer side under the remaining QKV stream
                            attn_tail(1, psA)
                            attn_begin(2)
                            pend2.append(attn_step(2, 0, psA))
                        if u == 30:
                            pend2.append(attn_step(2, 1, psA))
                    if sc == 2:
                        rope_vt(2)
                    base += 16
                rope_vt(3)
                for jt in range(12, 16):
                    nc.sync.dma_start_transpose(vn[:, jt, :], vT[:, jt * P:(jt + 1) * P])

            # ---- Phase B: attn(2/3) pipelined + dense ----
            def make_dense_units(pool):
                def dense_units(c, tail):
                    for st in range(4):
                        ot = outp.tile([P, 4, FD], bf16, tag="ot", name=f"ot_{c}_{st}")
                        for eo in range(4):
                            op = pool.tile([P, FD], f32, tag="o", name=f"o_{c}_{st}_{eo}")
                            for h in range(2):
                                nc.tensor.matmul(
                                    op,
                                    lhsT=all_csb[(c, h)][:, st * P:(st + 1) * P],
                                    rhs=wd_sb[:, h, eo * FD:(eo + 1) * FD],
                                    start=(h == 0), stop=(h == 1),
                                )
                            if (eo % 2) if tail else (eo == 3):
                                nc.scalar.copy(ot[:, eo, :], op)
                            else:
                                nc.vector.tensor_copy(ot[:, eo, :], op)
                            if tail and c == 3 and st == 3:
                                nc.sync.dma_start(out[c, st, :, eo, :], ot[:, eo, :])
                            elif eo == 3:
                                nc.sync.dma_start(out[c, st], ot)
                            yield
                return dense_units

            with tc.tile_pool(name="ps_sB", bufs=2, space="PSUM") as psB, \
                 tc.tile_pool(name="ps_o", bufs=2, space="PSUM") as ps_o:
                dense_units = make_dense_units(ps_o)

                def emit_attn_B(c, dq, pend=None, jstart=0, leave=0):
                    nj = 4 * c + 4
                    if pend is None:
                        attn_begin(c)
                        pend = []
                    nd = 0
                    for j in range(jstart, nj):
                        pend.append(attn_step(c, j, psB))
                        if dq is not None:
                            want = (j + 1 - jstart) * 16 // (nj - jstart)
                            while nd < want:
                                next(dq)
                                nd += 1
                        if len(pend) >= 3:
                            attn_ctx(c, pend.pop(0))
                    while len(pend) > leave:
                        attn_ctx(c, pend.pop(0))
                    if dq is not None:
                        for _ in dq:
                            pass
                    return pend

                emit_attn_B(2, dense_units(0, False), pend=pend2, jstart=2)
                attn_tail(2, psB)
                # keep ctx j14/j15 pending so the per-st finalize can
                # interleave with the drain (ctx j12/j13 emit in-loop)
                pend3 = emit_attn_B(3, dense_units(1, False), leave=2)
            with tc.tile_pool(name="ps_d", bufs=4, space="PSUM") as ps_d:
                dense_tail = make_dense_units(ps_d)

                def dense3_st(stt, last):
                    # chunk-3 dense per st-slice; copies alternate engines
                    # (baseline tail mix) so neither queue serializes
                    ot = outp.tile([P, 4, FD], bf16, tag="ot", name=f"o3t_{stt}")
                    for eo in range(4):
                        op = ps_d.tile([P, FD], f32, tag="o", name=f"o3_{stt}_{eo}")
                        for h in range(2):
                            nc.tensor.matmul(
                                op,
                                lhsT=all_csb[(3, h, stt)],
                                rhs=wd_sb[:, h, eo * FD:(eo + 1) * FD],
                                start=(h == 0), stop=(h == 1),
                            )
                        if eo % 2:
                            nc.scalar.copy(ot[:, eo, :], op)
                        else:
                            nc.vector.tensor_copy(ot[:, eo, :], op)
                        if last:
                            # final tile: drain over both HWDGE rings
                            eng = nc.sync if eo % 2 == 0 else nc.scalar
                            eng.dma_start(out[3, stt, :, eo, :], ot[:, eo, :])
                        elif eo == 3:
                            nc.sync.dma_start(out[3, stt], ot)

                # dense(2) streams on the PE while chunk 3's per-st softmax
                # finalize chains drain; dense(3, st) starts as soon as its
                # slice is final
                dq2 = dense_tail(2, True)
                for _ in range(4):
                    next(dq2)
                fin_st(3, 0, ps_d, "o")              # needs ctx j12 (in-loop)
                for _ in range(2):
                    next(dq2)
                fin_st(3, 1, ps_d, "o")              # needs ctx j13 (in-loop)
                for _ in range(2):
                    next(dq2)
                attn_ctx(3, pend3.pop(0))            # ctx j14
                fin_st(3, 2, ps_d, "o")
                dense3_st(0, False)
                for _ in range(2):
                    next(dq2)
                dense3_st(1, False)
                attn_ctx(3, pend3.pop(0))            # ctx j15 (stop)
                fin_st(3, 3, ps_d, "o")
                dense3_st(2, False)
                for _ in dq2:
                    pass
                dense3_st(3, True)
    nc.compile()
    return nc


def make_in_maps(x, w_qkv, w_dense):
    x = np.asarray(x, np.float32).reshape(S, E)
    w_qkv = np.asarray(w_qkv, np.float32)
    w_dense = np.asarray(w_dense, np.float32)
    # x^T tiled to [sc, g, p, j, f] so each 512KB DMA block is contiguous
    xT = np.ascontiguousarray(x.T)
    xG = np.ascontiguousarray(
        xT.reshape(NG, 4, P, NSC, FD).transpose(3, 0, 2, 1, 4)
    ).astype(BF)
    consts = _host_constants()
    in_maps = []
    scale = np.float64(1.0 / np.sqrt(D))
    for d in range(NCORES):
        g = d // 2
        wq = w_qkv[2 * d * P:(2 * d + 2) * P] * scale
        wk = w_qkv[H * D + g * P: H * D + (g + 1) * P]
        wv = w_qkv[H * D + KVH * D + g * P: H * D + KVH * D + (g + 1) * P]
        wqkvT_d = np.ascontiguousarray(np.concatenate([wq, wk, wv], 0).T)
        wG_d = np.ascontiguousarray(
            wqkvT_d.reshape(NG, 4, P, FLOC).transpose(0, 2, 1, 3)
        ).astype(BF)
        wdT_d = w_dense[:, 2 * d * P:(2 * d + 2) * P].T  # [2P, S]
        wdG_d = np.ascontiguousarray(
            wdT_d.reshape(2, P, S).transpose(1, 0, 2)
        ).astype(BF)
        m = {"xG": xG, "wG": wG_d, "wdG": wdG_d}
        m.update(consts)
        in_maps.append(m)
    return in_maps


def kernel(x, w_qkv, w_dense):
    global LAST_RESULT, _BASS_CACHE
    from concourse.bass_utils import run_bass_kernel_spmd

    in_maps = make_in_maps(x, w_qkv, w_dense)
    if _BASS_CACHE is None:
        _BASS_CACHE = _build_bass()
    res = run_bass_kernel_spmd(_BASS_CACHE, in_maps, core_ids=list(range(NCORES)))
    LAST_RESULT = res
    # sum partials over cores; [c, st, p, eo, f] flattens straight to [s, e]
    acc = np.zeros((NSC, 4, P, 4, FD), np.float32)
    for r in res.results:
        acc += r["out"].astype(np.float32)
    return np.ascontiguousarray(acc.reshape(S, E)).reshape(B, S, E)

